# revision 1
# baseline (speedup 1.0000x reference)
"""Cross-attention (GQA) Trainium2 Bass kernel.

Problem: B=2, Tq=Tkv=2048, D_MODEL=1024, 16 query heads / 4 kv heads,
head_dim=64.  Sharded over 8 NeuronCores as batch(2) x kv-group(4); each
core computes 4 query heads + its single kv head and a partial output
projection (Wo row-split by head group); partials are summed on host.

On-chip dataflow keeps activations "transposed" (feature dim on SBUF
partitions) end-to-end so that scores, softmax and P@V need no on-chip
transposes of large tensors:

  A: qT[e,t] = WqT.T @ xqT,  kvT = WkvT.T @ xcT        (fp32r, N=512)
     v[tk,dv] via PE-transpose of vT tiles
  B: ST[tk,tq] = kT.T @ qT_h ; two heads packed in the PE array via
     row-groups (K=64 each, h_even rows 0-63, h_odd rows 64-127)
  C: P = exp(ST/8)  on ScalarE, PSUM->SBUF, 1024-wide instructions
  D: outT'[dv+sum,tq] = [v|1].T @ P ; the ones-column matmul is
     col-packed into a spare PE column-group => denominators come out
     of the same pass.  h_odd heads are placed at partitions 64..127.
  E: yT += WoT_pair.T @ outT_norm (K=128: two heads stacked)
"""

import os
import sys

import numpy as np

for _p in ("/opt/trn_rl_repo",):
    if _p not in sys.path and os.path.isdir(_p):
        sys.path.insert(0, _p)

import concourse.bass as bass
import concourse.bacc as bacc
import concourse.mybir as mybir
from concourse.tile import TileContext

# ---------------------------------------------------------------- problem dims
B = 2
TQ = 2048
TKV = 2048
D_MODEL = 1024
N_HEADS = 16
N_KV_HEADS = 4
HEAD_DIM = 64
N_CORES = 8
GROUPS = N_KV_HEADS  # kv groups = 4
HEADS_PER_DEV = N_HEADS // GROUPS  # 4
DQ = HEADS_PER_DEV * HEAD_DIM  # 256
DKV = 2 * HEAD_DIM  # 128 (k rows + v rows stacked)
SCALE = 1.0 / float(np.sqrt(HEAD_DIM))

P = 128
FREE = 512  # matmul moving-operand chunk
BLK = 1024  # tq block width (exp instruction width)

F32 = mybir.dt.float32
F32R = mybir.dt.float32r
F16 = mybir.dt.float16


def build_bass():
    nc = bacc.Bacc()

    xq = nc.declare_dram_parameter("xqT", [D_MODEL, TQ], F16, isOutput=False)
    xc = nc.declare_dram_parameter("xcT", [D_MODEL, TKV], F16, isOutput=False)
    wq = nc.declare_dram_parameter("wqT", [D_MODEL, DQ], F16, isOutput=False)
    wkv = nc.declare_dram_parameter("wkvT", [D_MODEL, DKV], F16, isOutput=False)
    wo = nc.declare_dram_parameter("woT", [DQ, D_MODEL], F16, isOutput=False)
    cid = nc.declare_dram_parameter("cid", [P, P + 64], F16, isOutput=False)
    yt = nc.declare_dram_parameter("yT", [D_MODEL, TQ], F32, isOutput=True)

    DT = D_MODEL // P  # 8 d-tiles
    ET = DQ // P  # 2 e-tiles (query head pairs)
    NCH = TQ // FREE  # 4 chunks of 512
    NTK = TKV // P  # 16 tk tiles
    NBLK = TQ // BLK  # 2 tq blocks
    JPB = BLK // FREE  # 2 free-chunks per block
    MT = D_MODEL // P  # 8 output m-tiles

    with TileContext(nc) as tc:
        with (
            tc.tile_pool(name="consts", bufs=1) as consts,
            tc.tile_pool(name="xch", bufs=3) as xpool,
            tc.tile_pool(name="pt", bufs=6) as ptpool,
            tc.tile_pool(name="nrm", bufs=2) as nrmpool,
            tc.tile_pool(name="yout", bufs=3) as ypool,
            tc.tile_pool(name="psA", bufs=2, space="PSUM") as psA,
            tc.tile_pool(name="psB", bufs=2, space="PSUM") as psB,
        ):
            # ---------------- constants / persistent tiles
            ident = consts.tile([P, P + 64], F16, tag="ident")
            nc.sync.dma_start(ident, cid[:])
            ones = ident[:, P : P + 64]

            wq_sb = consts.tile([P, DT, DQ], F16, tag="wq")
            nc.sync.dma_start(wq_sb, wq.rearrange("(i p) e -> p i e", p=P))
            wkv_sb = consts.tile([P, DT, DKV], F16, tag="wkv")
            nc.sync.dma_start(wkv_sb, wkv.rearrange("(i p) e -> p i e", p=P))
            wo_sb = consts.tile([P, ET, D_MODEL], F16, tag="wo")
            nc.sync.dma_start(wo_sb, wo.rearrange("(i p) m -> p i m", p=P))

            qt = consts.tile([P, ET, TQ], F16, tag="qt")  # qT: heads 2/tile
            kv = consts.tile([P, TKV], F16, tag="kv")  # rows 0-63 kT, 64-127 vT
            k2 = consts.tile([P, TKV], F16, tag="k2")  # rows 64-127 = kT copy
            vp = consts.tile([P, NTK, P], F16, tag="vp")  # [v | ones]
            vp2 = consts.tile([P, NTK, P], F16, tag="vp2")  # [ones | v]
            outs = consts.tile([P, ET, TQ], F16, tag="outs")  # normalized outT

            # ---------------- stage A: projections (weights stationary)
            # kv first (every BCD iteration needs the full kT/vT), then q
            for c in range(NCH):
                cs = slice(c * FREE, (c + 1) * FREE)
                xc_t = xpool.tile([P, DT, FREE], F16, tag="xch")
                nc.sync.dma_start(
                    xc_t, xc.rearrange("(i p) t -> p i t", p=P)[:, :, cs]
                )
                pkv = psB.tile([P, FREE], F32, tag="psB")
                for i in range(DT):
                    nc.tensor.matmul(
                        pkv,
                        (wkv_sb[:, i, :]),
                        (xc_t[:, i, :]),
                        start=(i == 0),
                        stop=(i == DT - 1),
                    )
                nc.vector.tensor_copy(kv[:, cs], pkv)
                # duplicate kT rows into partitions 64..127 for row-packing
                nc.sync.dma_start(k2[HEAD_DIM : 2 * HEAD_DIM, cs], kv[:HEAD_DIM, cs])

            def emit_q_chunk(c):
                cs = slice(c * FREE, (c + 1) * FREE)
                xq_t = xpool.tile([P, DT, FREE], F16, tag="xch", name="xq_t")
                nc.sync.dma_start(
                    xq_t, xq.rearrange("(i p) t -> p i t", p=P)[:, :, cs]
                )
                for e in range(ET):
                    pq = psA.tile([P, FREE], F32, tag="psA", name="pq")
                    for i in range(DT):
                        nc.tensor.matmul(
                            pq,
                            (wq_sb[:, i, e * P : (e + 1) * P]),
                            (xq_t[:, i, :]),
                            start=(i == 0),
                            stop=(i == DT - 1),
                        )
                    nc.vector.tensor_copy(qt[:, e, cs], pq)

            for _c in range(min(2, NCH)):
                emit_q_chunk(_c)

            # v' tiles: PE-transpose vT[64, tk*128 ..] -> [128, 64], then
            # build [v | ones] (for even heads) and [ones | v] (odd heads).
            # The all-ones half makes the same matmul emit the softmax
            # denominators, replicated across 64 partitions.
            for t in range(NTK):
                ts_ = slice(t * P, (t + 1) * P)
                pv = psB.tile([P, HEAD_DIM], F16, tag="psB")
                nc.tensor.transpose(
                    pv, kv[HEAD_DIM : 2 * HEAD_DIM, ts_], ident[HEAD_DIM:, HEAD_DIM:P]
                )
                nc.vector.tensor_copy(vp[:, t, :HEAD_DIM], pv)
                nc.vector.tensor_copy(vp2[:, t, HEAD_DIM:], pv)
                nc.vector.tensor_copy(vp[:, t, HEAD_DIM:], ones)
                nc.vector.tensor_copy(vp2[:, t, :HEAD_DIM], ones)

            # -------- stage E chunk emitter (interleaved into BCD stream)
            def emit_out_chunk(c):
                cs = slice(c * FREE, (c + 1) * FREE)
                for m in range(MT):
                    ms = slice(m * P, (m + 1) * P)
                    py = psA.tile([P, FREE], F32, tag="psA", name="py")
                    for ee in range(ET):
                        nc.tensor.matmul(
                            py,
                            (wo_sb[:, ee, ms]),
                            (outs[:, ee, cs]),
                            start=(ee == 0),
                            stop=(ee == ET - 1),
                        )
                    yo = ypool.tile([P, FREE], F32, tag="yout", name="yo")
                    nc.vector.tensor_copy(yo, py)
                    nc.sync.dma_start(yt[ms, cs], yo)

            # ---------------- stages B/C/D: attention per head-pair
            first_bcd = True
            for blk in range(NBLK):
                for e in range(ET):  # head pair (h_even=2e, h_odd=2e+1)
                    bs = slice(blk * BLK, (blk + 1) * BLK)
                    pd = [
                        psB.tile([P, BLK], F32, tag="psB", name=f"pd{_h}")
                        for _h in range(2)
                    ]  # D accumulators: [0]=h_even rows 0-64, [1]=h_odd
                    for t in range(NTK):
                        ts_ = slice(t * P, (t + 1) * P)
                        pb = [
                            psA.tile([P, BLK], F32, tag="psA", name=f"pb{_h}")
                            for _h in range(2)
                        ]
                        for j in range(JPB):
                            js = slice(blk * BLK + j * FREE, blk * BLK + (j + 1) * FREE)
                            jo = slice(j * FREE, (j + 1) * FREE)
                            # scores, 2 heads row-packed (K=64 each)
                            nc.tensor.matmul(
                                pb[0][:, jo],
                                (kv[:HEAD_DIM, ts_]),
                                (qt[:HEAD_DIM, e, js]),
                            )
                            nc.tensor.matmul(
                                pb[1][:, jo],
                                (k2[HEAD_DIM:, ts_]),
                                (qt[HEAD_DIM:, e, js]),
                            )
                        for h in range(2):
                            pt = ptpool.tile([P, BLK], F16, tag="pt")
                            nc.scalar.activation(
                                pt,
                                pb[h],
                                mybir.ActivationFunctionType.Exp,
                                bias=0.0,
                                scale=SCALE,
                            )
                            # M=128 stationary [v|ones] / [ones|v]: one
                            # matmul per head yields out_h in its 64-row
                            # half and the softmax denominators (replicated
                            # x64) in the other half.  dst base stays 0
                            # (fp32r matmuls cannot target offset psum
                            # partitions).
                            vo = vp if h == 0 else vp2
                            for j in range(JPB):
                                jo = slice(j * FREE, (j + 1) * FREE)
                                nc.tensor.matmul(
                                    pd[h][:, jo],
                                    vo[:, t, :],
                                    pt[:, jo],
                                    start=(t == 0),
                                    stop=(t == NTK - 1),
                                    skip_group_check=True,
                                )
                    if first_bcd:
                        first_bcd = False
                        for _c in range(2, NCH):
                            emit_q_chunk(_c)
                    # spill raw accumulators to SBUF immediately (~1.2us)
                    # so the PSUM slots free up and the PE never stalls;
                    # the normalize chain below runs off the critical path.
                    for h in range(2):
                        raw = nrmpool.tile([P, BLK], F32, tag=f"raw{h}")
                        nc.vector.tensor_copy(raw, pd[h])
                        lo = slice(0, 64) if h == 0 else slice(64, 128)
                        hi = slice(64, 128) if h == 0 else slice(0, 64)
                        rec = nrmpool.tile([P, BLK], F32, tag="rec")
                        rec2 = nrmpool.tile([P, BLK], F32, tag="rec2")
                        nc.vector.reciprocal(rec[hi, :], raw[hi, :])
                        nc.sync.dma_start(rec2[lo, :], rec[hi, :])
                        nc.vector.tensor_mul(
                            outs[lo, e, bs], raw[lo, :], rec2[lo, :]
                        )
                    if e == ET - 1:
                        for _c in range(blk * (BLK // FREE), (blk + 1) * (BLK // FREE)):
                            emit_out_chunk(_c)


    nc.finalize()  # Bacc: runs wait-splitting/reg-alloc passes
    return nc


_NC_CACHE = None


def _get_nc():
    global _NC_CACHE
    if _NC_CACHE is None:
        _NC_CACHE = build_bass()
    return _NC_CACHE


def _cid():
    c = np.zeros((P, P + 64), dtype=np.float16)
    c[:, :P] = np.eye(P, dtype=np.float32)
    c[:, P:] = 1.0
    return c


def shard_inputs(query, context, Wq, Wk, Wv, Wo):
    """host-side sharding: 8 cores = batch(2) x kv-group(4)"""
    in_maps = []
    xqT = [np.ascontiguousarray(query[b].T).astype(np.float16) for b in range(B)]
    xcT = [np.ascontiguousarray(context[b].T).astype(np.float16) for b in range(B)]
    for core in range(N_CORES):
        b, g = divmod(core, GROUPS)
        wqT = np.ascontiguousarray(Wq[g * DQ : (g + 1) * DQ, :].T).astype(np.float16)
        wkvT = np.ascontiguousarray(
            np.concatenate(
                [
                    Wk[g * HEAD_DIM : (g + 1) * HEAD_DIM, :],
                    Wv[g * HEAD_DIM : (g + 1) * HEAD_DIM, :],
                ],
                axis=0,
            ).T
        ).astype(np.float16)
        woT = np.ascontiguousarray(Wo[:, g * DQ : (g + 1) * DQ].T).astype(np.float16)
        in_maps.append(
            {
                "xqT": xqT[b],
                "xcT": xcT[b],
                "wqT": wqT,
                "wkvT": wkvT,
                "woT": woT,
                "cid": _cid(),
            }
        )
    return in_maps


def kernel(query, context, Wq, Wk, Wv, Wo, _want_profile=False):
    from concourse.bass_utils import run_bass_kernel_spmd

    nc = _get_nc()
    in_maps = shard_inputs(query, context, Wq, Wk, Wv, Wo)
    res = run_bass_kernel_spmd(
        nc, in_maps, core_ids=list(range(N_CORES)), trace=_want_profile
    )
    out = np.zeros((B, TQ, D_MODEL), dtype=np.float32)
    for core in range(N_CORES):
        b = core // GROUPS
        out[b] += res.results[core]["yT"].T
    if _want_profile:
        return out, res
    return out



# revision 7
# speedup vs baseline: 1.2244x; 1.2244x over previous
"""Cross-attention (GQA) Trainium2 Bass kernel.

Problem: B=2, Tq=Tkv=2048, D_MODEL=1024, 16 query heads / 4 kv heads,
head_dim=64.  Sharded over 8 NeuronCores as batch(2) x kv-group(4); each
core computes 4 query heads + its single kv head and a partial output
projection (Wo row-split by head group); partials are summed on host.

Dataflow (all activations kept feature-major / "transposed" on chip):

  A: qT[e,t] = WqT.T @ xqT,  kvT = WkvT.T @ xcT      (fp16, N=1024 chunks)
     v tiles built via DMA-xbar transpose of vT (no PE/DVE involvement)
  B: ST[tk,tq] = kT.T @ qT_h ; the two heads of an e-pair issue
     back-to-back as K=64 row-group-tiled matmuls -> run concurrently
  C: P = exp(ST/8): split between ScalarE (true exp LUT) and VectorE
     (1-instruction Schraudolph: fp16 bits = int16(A*s + B), ~3% elem
     error that averages out ~1/sqrt(750) under the softmax weighting)
  D: outT'[dv|sum, tq] = [v|1].T @ P ; ones-columns give the softmax
     denominators in the spare output rows for free.  D for tile t is
     emitted 2 iterations behind B(t) so the PE never waits on exp.
  norm: reciprocal_approx_fast + cross-partition DMA + multiply
  E: yT = WoT_pair.T @ outs as a dense tail phase, f16 DMA out.
"""

import os
import sys

import numpy as np

for _p in ("/opt/trn_rl_repo",):
    if _p not in sys.path and os.path.isdir(_p):
        sys.path.insert(0, _p)

import concourse.bass as bass
import concourse.bacc as bacc
import concourse.mybir as mybir
from concourse.tile import TileContext

# ---------------------------------------------------------------- problem dims
B = 2
TQ = 2048
TKV = 2048
D_MODEL = 1024
N_HEADS = 16
N_KV_HEADS = 4
HEAD_DIM = 64
N_CORES = 8
GROUPS = N_KV_HEADS  # kv groups = 4
HEADS_PER_DEV = N_HEADS // GROUPS  # 4
DQ = HEADS_PER_DEV * HEAD_DIM  # 256
DKV = 2 * HEAD_DIM  # 128 (k rows + v rows stacked)
SCALE = 1.0 / float(np.sqrt(HEAD_DIM))

P = 128
FREE = 512  # PSUM-bank-limited matmul N (fp32 out)
CHK = 512  # projection / out-proj moving chunk (PSUM one-bank limit)
BLK = 1024  # tq block processed per BCD unit

F32 = mybir.dt.float32
F16 = mybir.dt.float16
I16 = mybir.dt.int16

# Schraudolph exp in fp16-bit domain: f16bits(e^(s/8)) ~ int16(EXPA*s + EXPB)
EXPA = float(SCALE * np.log2(np.e) * 1024.0)  # 184.664
EXPB = 15315.5  # 15360 - 44.5 (minimax-centered)

# exp-tile engine assignment: of every 32 exp tiles, this many go to ScalarE
ACT_OF_32 = 17


def _use_dve_exp(ctr: int) -> bool:
    return (ctr % 32) >= ACT_OF_32


def build_bass():
    nc = bacc.Bacc()

    xq = nc.declare_dram_parameter("xqT", [D_MODEL, TQ], F16, isOutput=False)
    xc = nc.declare_dram_parameter("xcT", [D_MODEL, TKV], F16, isOutput=False)
    wq = nc.declare_dram_parameter("wqT", [D_MODEL, DQ], F16, isOutput=False)
    wkv = nc.declare_dram_parameter("wkvT", [D_MODEL, DKV], F16, isOutput=False)
    wo = nc.declare_dram_parameter("woT", [DQ, D_MODEL], F16, isOutput=False)
    yt = nc.declare_dram_parameter("yT", [D_MODEL, TQ], F16, isOutput=True)

    DT = D_MODEL // P  # 8 contraction tiles
    ET = DQ // P  # 2 q-head-pair tiles
    NTK = TKV // P  # 16 tk tiles
    NBLK = TQ // BLK  # 2 tq blocks
    JPB = BLK // FREE  # 2 free-chunks per block
    NCH = TQ // CHK  # 2 chunks of 1024 for A/E

    with TileContext(nc) as tc:
        with (
            tc.tile_pool(name="consts", bufs=1) as consts,
            tc.tile_pool(name="xch", bufs=4) as xpool,
            tc.tile_pool(name="pt", bufs=12) as ptpool,
            tc.tile_pool(name="nrm", bufs=2) as nrmpool,
            tc.tile_pool(name="ys", bufs=3) as yspool,
            tc.tile_pool(name="psA", bufs=4, space="PSUM") as psA,
            tc.tile_pool(name="psB", bufs=2, space="PSUM") as psB,
        ):
            # ---------------- persistent tiles
            wq_sb = consts.tile([P, DT, DQ], F16, tag="wq")
            nc.sync.dma_start(wq_sb, wq.rearrange("(i p) e -> p i e", p=P))
            wkv_sb = consts.tile([P, DT, DKV], F16, tag="wkv")
            nc.sync.dma_start(wkv_sb, wkv.rearrange("(i p) e -> p i e", p=P))
            wo_sb = consts.tile([P, ET, D_MODEL], F16, tag="wo")
            nc.sync.dma_start(wo_sb, wo.rearrange("(i p) m -> p i m", p=P))

            qt = consts.tile([P, ET, TQ], F16, tag="qt")  # qT: head pair/tile
            kv = consts.tile([P, TKV], F16, tag="kv")  # rows 0-63 kT, 64-127 vT
            k2 = consts.tile([P, TKV], F16, tag="k2")  # rows 64-127 = kT copy
            vp = consts.tile([P, NTK, P], F16, tag="vp")  # [v | ones]
            vp2 = consts.tile([P, NTK, P], F16, tag="vp2")  # [ones | v]
            outs = consts.tile([P, ET, TQ], F16, tag="outs")  # normalized outT

            # ones halves of the [v|1] stationaries, written once
            nc.vector.memset(vp[:, :, HEAD_DIM:], 1.0)
            nc.vector.memset(vp2[:, :, :HEAD_DIM], 1.0)

            # ---------------- stage A: kv projection (4 chunks of 512,
            # i-outer / chunk-inner so each LDWEIGHTS serves 4 matmuls)
            xc_t = []
            for c in range(NCH):
                t_ = xpool.tile([P, DT, CHK], F16, tag="xch", name=f"xc{c}")
                cs = slice(c * CHK, (c + 1) * CHK)
                nc.sync.dma_start(t_, xc.rearrange("(i p) t -> p i t", p=P)[:, :, cs])
                xc_t.append(t_)
            pkv = [
                psA.tile([P, CHK], F32, tag="pb", name=f"pkv{c}") for c in range(NCH)
            ]
            for i in range(DT):
                for c in range(NCH):
                    nc.tensor.matmul(
                        pkv[c],
                        wkv_sb[:, i, :],
                        xc_t[c][:, i, :],
                        start=(i == 0),
                        stop=(i == DT - 1),
                    )
            for c in range(NCH):
                cs = slice(c * CHK, (c + 1) * CHK)
                nc.scalar.copy(kv[:, cs], pkv[c])
            # duplicate kT rows into partitions 64..127 for row-group packing
            nc.sync.dma_start(k2[HEAD_DIM : 2 * HEAD_DIM, :], kv[:HEAD_DIM, :])
            # v tiles via DMA-xbar transpose: [64,128] slabs -> [128,64]
            for t in range(NTK):
                ts_ = slice(t * P, (t + 1) * P)
                nc.sync.dma_start_transpose(
                    vp[:, t, :HEAD_DIM], kv[HEAD_DIM : 2 * HEAD_DIM, ts_]
                )
                nc.sync.dma_start_transpose(
                    vp2[:, t, HEAD_DIM:], kv[HEAD_DIM : 2 * HEAD_DIM, ts_]
                )

            # ---------------- stage A: q projection (chunk-pairs, e-major,
            # i-outer / chunk-inner so each LDWEIGHTS serves 2 matmuls)
            for half in range(NCH // 2):
                xq_t = []
                for c2 in range(2):
                    c = half * 2 + c2
                    t_ = xpool.tile([P, DT, CHK], F16, tag="xch", name=f"xq{c}")
                    cs = slice(c * CHK, (c + 1) * CHK)
                    nc.sync.dma_start(
                        t_, xq.rearrange("(i p) t -> p i t", p=P)[:, :, cs]
                    )
                    xq_t.append(t_)
                for e in range(ET):
                    pq = [
                        psA.tile([P, CHK], F32, tag="pb", name=f"pq{e}{c2}")
                        for c2 in range(2)
                    ]
                    for i in range(DT):
                        for c2 in range(2):
                            nc.tensor.matmul(
                                pq[c2],
                                wq_sb[:, i, e * P : (e + 1) * P],
                                xq_t[c2][:, i, :],
                                start=(i == 0),
                                stop=(i == DT - 1),
                            )
                    for c2 in range(2):
                        cs = slice(
                            (half * 2 + c2) * CHK, (half * 2 + c2 + 1) * CHK
                        )
                        nc.vector.tensor_copy(qt[:, e, cs], pq[c2])

            # ---------------- stages B/C/D, software-pipelined per tk tile
            expctr = 0
            for blk in range(NBLK):
                for e in range(ET):
                    bs = slice(blk * BLK, (blk + 1) * BLK)
                    pd = [
                        psB.tile([P, BLK], F32, tag="pd", name=f"pd{_h}")
                        for _h in range(2)
                    ]

                    def emit_D(item):
                        t, pts = item
                        for h in range(2):
                            vo = vp if h == 0 else vp2
                            for j in range(JPB):
                                jo = slice(j * FREE, (j + 1) * FREE)
                                nc.tensor.matmul(
                                    pd[h][:, jo],
                                    vo[:, t, :],
                                    pts[(h, j)],
                                    start=(t == 0),
                                    stop=(t == NTK - 1),
                                    skip_group_check=True,
                                )

                    pending = []
                    for t in range(NTK):
                        ts_ = slice(t * P, (t + 1) * P)
                        pb = {}
                        for j in range(JPB):
                            js = slice(
                                blk * BLK + j * FREE, blk * BLK + (j + 1) * FREE
                            )
                            for h in range(2):
                                pb[(h, j)] = psA.tile(
                                    [P, FREE], F32, tag="pb", name=f"pb{h}{j}"
                                )
                            # adjacent issue => concurrent K=64 row-group pair
                            nc.tensor.matmul(
                                pb[(0, j)], kv[:HEAD_DIM, ts_], qt[:HEAD_DIM, e, js]
                            )
                            nc.tensor.matmul(
                                pb[(1, j)],
                                k2[HEAD_DIM:, ts_],
                                qt[HEAD_DIM:, e, js],
                            )
                        pts = {}
                        for h in range(2):
                            for j in range(JPB):
                                pt = ptpool.tile([P, FREE], F16, tag="pt")
                                if _use_dve_exp(expctr):
                                    nc.vector.tensor_scalar(
                                        pt.bitcast(I16),
                                        pb[(h, j)],
                                        EXPA,
                                        EXPB,
                                        mybir.AluOpType.mult,
                                        mybir.AluOpType.add,
                                    )
                                else:
                                    nc.scalar.activation(
                                        pt,
                                        pb[(h, j)],
                                        mybir.ActivationFunctionType.Exp,
                                        bias=0.0,
                                        scale=SCALE,
                                    )
                                expctr += 1
                                pts[(h, j)] = pt
                        pending.append((t, pts))
                        if len(pending) > 2:
                            emit_D(pending.pop(0))
                    for item in pending:
                        emit_D(item)

                    # normalize: raw spill, approx-recip of the denominator
                    # rows, cross-partition DMA, multiply into `outs`
                    for h in range(2):
                        raw = nrmpool.tile([P, BLK], F32, tag=f"raw{h}")
                        if h == 0:
                            nc.scalar.copy(raw, pd[h])
                        else:
                            nc.vector.tensor_copy(raw, pd[h])
                        lo = slice(0, 64) if h == 0 else slice(64, 128)
                        hi = slice(64, 128) if h == 0 else slice(0, 64)
                        rec = nrmpool.tile([P, BLK], F32, tag="rec")
                        rec2 = nrmpool.tile([P, BLK], F32, tag="rec2")
                        nc.vector.reciprocal_approx_fast(rec[hi, :], raw[hi, :])
                        nc.sync.dma_start(rec2[lo, :], rec[hi, :])
                        nc.vector.tensor_mul(outs[lo, e, bs], raw[lo, :], rec2[lo, :])

            # ---------------- stage E: out-projection tail, f16 DMA out
            for c in range(NCH):
                cs = slice(c * CHK, (c + 1) * CHK)
                for m in range(DT):
                    ms = slice(m * P, (m + 1) * P)
                    py = psA.tile([P, CHK], F32, tag="pb", name="py")
                    for ee in range(ET):
                        nc.tensor.matmul(
                            py,
                            wo_sb[:, ee, ms],
                            outs[:, ee, cs],
                            start=(ee == 0),
                            stop=(ee == ET - 1),
                        )
                    ys = yspool.tile([P, CHK], F16, tag="ys", name="ys")
                    if m % 2 == 0:
                        nc.scalar.copy(ys, py)
                    else:
                        nc.vector.tensor_copy(ys, py)
                    nc.sync.dma_start(yt[ms, cs], ys)

    nc.finalize()  # Bacc: runs wait-splitting/reg-alloc passes
    return nc


_NC_CACHE = None


def _get_nc():
    global _NC_CACHE
    if _NC_CACHE is None:
        _NC_CACHE = build_bass()
    return _NC_CACHE


def shard_inputs(query, context, Wq, Wk, Wv, Wo):
    """host-side sharding: 8 cores = batch(2) x kv-group(4)"""
    in_maps = []
    xqT = [np.ascontiguousarray(query[b].T).astype(np.float16) for b in range(B)]
    xcT = [np.ascontiguousarray(context[b].T).astype(np.float16) for b in range(B)]
    for core in range(N_CORES):
        b, g = divmod(core, GROUPS)
        wqT = np.ascontiguousarray(Wq[g * DQ : (g + 1) * DQ, :].T).astype(np.float16)
        wkvT = np.ascontiguousarray(
            np.concatenate(
                [
                    Wk[g * HEAD_DIM : (g + 1) * HEAD_DIM, :],
                    Wv[g * HEAD_DIM : (g + 1) * HEAD_DIM, :],
                ],
                axis=0,
            ).T
        ).astype(np.float16)
        woT = np.ascontiguousarray(Wo[:, g * DQ : (g + 1) * DQ].T).astype(np.float16)
        in_maps.append(
            {
                "xqT": xqT[b],
                "xcT": xcT[b],
                "wqT": wqT,
                "wkvT": wkvT,
                "woT": woT,
            }
        )
    return in_maps


def kernel(query, context, Wq, Wk, Wv, Wo, _want_profile=False):
    from concourse.bass_utils import run_bass_kernel_spmd

    nc = _get_nc()
    in_maps = shard_inputs(query, context, Wq, Wk, Wv, Wo)
    res = run_bass_kernel_spmd(
        nc, in_maps, core_ids=list(range(N_CORES)), trace=_want_profile
    )
    out = np.zeros((B, TQ, D_MODEL), dtype=np.float32)
    for core in range(N_CORES):
        b = core // GROUPS
        out[b] += res.results[core]["yT"].T.astype(np.float32)
    if _want_profile:
        return out, res
    return out


# revision 20
# speedup vs baseline: 1.3041x; 1.0651x over previous
"""Cross-attention (GQA) Trainium2 Bass kernel.

Problem: B=2, Tq=Tkv=2048, D_MODEL=1024, 16 query heads / 4 kv heads,
head_dim=64.  Sharded over 8 NeuronCores as batch(2) x kv-group(4); each
core computes 4 query heads + its single kv head and a partial output
projection (Wo row-split by head group); partials are summed on host.

Dataflow (all activations kept feature-major / "transposed" on chip):

  A: qT[e,t] = WqT.T @ xqT,  kvT = WkvT.T @ xcT      (fp16, N=1024 chunks)
     v tiles built via DMA-xbar transpose of vT (no PE/DVE involvement)
  B: ST[tk,tq] = kT.T @ qT_h ; the two heads of an e-pair issue
     back-to-back as K=64 row-group-tiled matmuls -> run concurrently
  C: P = exp(ST/8): split between ScalarE (true exp LUT) and VectorE
     (1-instruction Schraudolph: fp16 bits = int16(A*s + B), ~3% elem
     error that averages out ~1/sqrt(750) under the softmax weighting)
  D: outT'[dv|sum, tq] = [v|1].T @ P ; ones-columns give the softmax
     denominators in the spare output rows for free.  D for tile t is
     emitted 2 iterations behind B(t) so the PE never waits on exp.
  norm: reciprocal_approx_fast + cross-partition DMA + multiply
  E: yT = WoT_pair.T @ outs as a dense tail phase, f16 DMA out.
"""

import os
import sys

import numpy as np

for _p in ("/opt/trn_rl_repo",):
    if _p not in sys.path and os.path.isdir(_p):
        sys.path.insert(0, _p)

import concourse.bass as bass
import concourse.bacc as bacc
import concourse.mybir as mybir
from concourse.tile import TileContext

# ---------------------------------------------------------------- problem dims
B = 2
TQ = 2048
TKV = 2048
D_MODEL = 1024
N_HEADS = 16
N_KV_HEADS = 4
HEAD_DIM = 64
N_CORES = 8
GROUPS = N_KV_HEADS  # kv groups = 4
HEADS_PER_DEV = N_HEADS // GROUPS  # 4
DQ = HEADS_PER_DEV * HEAD_DIM  # 256
DKV = 2 * HEAD_DIM  # 128 (k rows + v rows stacked)
SCALE = 1.0 / float(np.sqrt(HEAD_DIM))

P = 128
FREE = 512  # PSUM-bank-limited matmul N (fp32 out)
CHK = 512  # projection / out-proj moving chunk (PSUM one-bank limit)
BLK = 1024  # tq block processed per BCD unit

F32 = mybir.dt.float32
F16 = mybir.dt.float16
I16 = mybir.dt.int16

# Schraudolph exp in fp16-bit domain: f16bits(e^(s/8)) ~ int16(EXPA*s + EXPB)
EXPA = float(os.environ.get("K_EXPA", SCALE * np.log2(np.e) * 1024.0))  # 184.664
EXPB = float(os.environ.get("K_EXPB", "15301.1"))  # 15360 - 58.9 (mean-zero)

# exp-tile engine split: DVE gets |DVE_TSET|/16 of tiles.  The index is
# (t + 8j + 4h) mod 16 so every softmax row sees exactly the same
# DVE fraction (max-error is governed by the worst row), and adjacent
# t alternate engines so ScalarE/VectorE run concurrently.
DVE_TSET = frozenset(
    int(x) for x in os.environ.get("K_DVET", "1,3,5,7,9,11,13").split(",") if x != ""
)
DEBUG = os.environ.get("K_DEBUG", "0") == "1"


def _use_dve_exp(t: int, h: int, j: int) -> bool:
    return ((t + 8 * j + 4 * h) % 16) in DVE_TSET


def build_bass():
    nc = bacc.Bacc()

    xq = nc.declare_dram_parameter("xqT", [D_MODEL, TQ], F16, isOutput=False)
    xc = nc.declare_dram_parameter("xcT", [D_MODEL, TKV], F16, isOutput=False)
    wq = nc.declare_dram_parameter("wqT", [D_MODEL, DQ], F16, isOutput=False)
    wkv = nc.declare_dram_parameter("wkvT", [D_MODEL, DKV], F16, isOutput=False)
    wo = nc.declare_dram_parameter("woT", [DQ, D_MODEL], F16, isOutput=False)
    yt = nc.declare_dram_parameter("yT", [D_MODEL, TQ], F16, isOutput=True)
    dbg = {}
    if DEBUG:
        for nm, shp in [
            ("dbg_kv", [P, TKV]),
            ("dbg_vp", [P, 16 * P]),
            ("dbg_vp2", [P, 16 * P]),
            ("dbg_qt", [P, 2 * TQ]),
            ("dbg_outs", [P, 2 * TQ]),
        ]:
            dbg[nm] = nc.declare_dram_parameter(nm, shp, F16, isOutput=True)
        for nm in ("dbg_raw", "dbg_rec2"):
            dbg[nm] = nc.declare_dram_parameter(nm, [P, BLK], F32, isOutput=True)

    DT = D_MODEL // P  # 8 contraction tiles
    ET = DQ // P  # 2 q-head-pair tiles
    NTK = TKV // P  # 16 tk tiles
    NBLK = TQ // BLK  # 2 tq blocks
    JPB = BLK // FREE  # 2 free-chunks per block
    NCH = TQ // CHK  # 2 chunks of 1024 for A/E

    with TileContext(nc) as tc:
        with (
            tc.tile_pool(name="consts", bufs=1) as consts,
            tc.tile_pool(name="xch", bufs=4) as xpool,
            tc.tile_pool(name="pt", bufs=12) as ptpool,
            tc.tile_pool(name="nrm", bufs=2) as nrmpool,
            tc.tile_pool(name="ys", bufs=3) as yspool,
            tc.tile_pool(name="psA", bufs=4, space="PSUM") as psA,
            tc.tile_pool(name="psB", bufs=2, space="PSUM") as psB,
        ):
            # ---------------- persistent tiles
            wq_sb = consts.tile([P, DT, DQ], F16, tag="wq")
            nc.sync.dma_start(wq_sb, wq.rearrange("(i p) e -> p i e", p=P))
            wkv_sb = consts.tile([P, DT, DKV], F16, tag="wkv")
            nc.sync.dma_start(wkv_sb, wkv.rearrange("(i p) e -> p i e", p=P))
            wo_sb = consts.tile([P, ET, D_MODEL], F16, tag="wo")
            nc.sync.dma_start(wo_sb, wo.rearrange("(i p) m -> p i m", p=P))

            qt = consts.tile([P, ET, TQ], F16, tag="qt")  # qT: head pair/tile
            kv = consts.tile([P, TKV], F16, tag="kv")  # rows 0-63 kT, 64-127 vT
            k2 = consts.tile([P, TKV], F16, tag="k2")  # rows 64-127 = kT copy
            vp = consts.tile([P, NTK, P], F16, tag="vp")  # [v | ones]
            vp2 = consts.tile([P, NTK, P], F16, tag="vp2")  # [ones | v]
            outs = consts.tile([P, ET, TQ], F16, tag="outs")  # normalized outT

            # ones halves of the [v|1] stationaries, written once
            nc.vector.memset(vp[:, :, HEAD_DIM:], 1.0)
            nc.vector.memset(vp2[:, :, :HEAD_DIM], 1.0)

            # ---------------- stage A: kv projection (4 chunks of 512,
            # i-outer / chunk-inner so each LDWEIGHTS serves 4 matmuls)
            xc_t = []
            for c in range(NCH):
                t_ = xpool.tile([P, DT, CHK], F16, tag="xch", name=f"xc{c}")
                cs = slice(c * CHK, (c + 1) * CHK)
                nc.sync.dma_start(t_, xc.rearrange("(i p) t -> p i t", p=P)[:, :, cs])
                xc_t.append(t_)
            pkv = [
                psA.tile([P, CHK], F32, tag="pb", name=f"pkv{c}") for c in range(NCH)
            ]
            for i in range(DT):
                for c in range(NCH):
                    nc.tensor.matmul(
                        pkv[c],
                        wkv_sb[:, i, :],
                        xc_t[c][:, i, :],
                        start=(i == 0),
                        stop=(i == DT - 1),
                    )
            for c in range(NCH):
                cs = slice(c * CHK, (c + 1) * CHK)
                nc.scalar.copy(kv[:, cs], pkv[c])
            # duplicate kT rows into partitions 64..127 for row-group packing
            # (Activation HWDGE queue: keeps the Sync queue free for the
            # xq chunk DMAs that gate the first BCD unit)
            nc.scalar.dma_start(k2[HEAD_DIM : 2 * HEAD_DIM, :], kv[:HEAD_DIM, :])
            # v tiles via DMA-xbar transpose, single 3D-output call:
            # vp[p, t, d] = vT[d, t*128+p]
            nc.scalar.dma_start_transpose(
                vp[:, :, :HEAD_DIM], kv[HEAD_DIM : 2 * HEAD_DIM, :]
            )
            nc.scalar.dma_start_transpose(
                vp2[:, :, HEAD_DIM:], kv[HEAD_DIM : 2 * HEAD_DIM, :]
            )

            # ---------------- stage A: q projection (chunk-pairs, e-major,
            # i-outer / chunk-inner so each LDWEIGHTS serves 2 matmuls)
            for half in range(NCH // 2):
                xq_t = []
                for c2 in range(2):
                    c = half * 2 + c2
                    t_ = xpool.tile([P, DT, CHK], F16, tag="xch", name=f"xq{c}")
                    cs = slice(c * CHK, (c + 1) * CHK)
                    nc.sync.dma_start(
                        t_, xq.rearrange("(i p) t -> p i t", p=P)[:, :, cs]
                    )
                    xq_t.append(t_)
                for e in range(ET):
                    pq = [
                        psA.tile([P, CHK], F32, tag="pb", name=f"pq{e}{c2}")
                        for c2 in range(2)
                    ]
                    for i in range(DT):
                        for c2 in range(2):
                            nc.tensor.matmul(
                                pq[c2],
                                wq_sb[:, i, e * P : (e + 1) * P],
                                xq_t[c2][:, i, :],
                                start=(i == 0),
                                stop=(i == DT - 1),
                            )
                    for c2 in range(2):
                        cs = slice(
                            (half * 2 + c2) * CHK, (half * 2 + c2 + 1) * CHK
                        )
                        nc.vector.tensor_copy(qt[:, e, cs], pq[c2])

            # ---------------- stages B/C/D, software-pipelined per tk tile
            expctr = 0
            for blk in range(NBLK):
                for e in range(ET):
                    bs = slice(blk * BLK, (blk + 1) * BLK)
                    pd = [
                        psB.tile([P, BLK], F32, tag="pd", name=f"pd{_h}")
                        for _h in range(2)
                    ]

                    def emit_D(item):
                        t, pts = item
                        for h in range(2):
                            vo = vp if h == 0 else vp2
                            for j in range(JPB):
                                jo = slice(j * FREE, (j + 1) * FREE)
                                nc.tensor.matmul(
                                    pd[h][:, jo],
                                    vo[:, t, :],
                                    pts[(h, j)],
                                    start=(t == 0),
                                    stop=(t == NTK - 1),
                                    skip_group_check=True,
                                )

                    pending = []
                    for t in range(NTK):
                        ts_ = slice(t * P, (t + 1) * P)
                        pb = {}
                        for j in range(JPB):
                            js = slice(
                                blk * BLK + j * FREE, blk * BLK + (j + 1) * FREE
                            )
                            for h in range(2):
                                pb[(h, j)] = psA.tile(
                                    [P, FREE], F32, tag="pb", name=f"pb{h}{j}"
                                )
                            # adjacent issue => concurrent K=64 row-group pair
                            nc.tensor.matmul(
                                pb[(0, j)], kv[:HEAD_DIM, ts_], qt[:HEAD_DIM, e, js]
                            )
                            nc.tensor.matmul(
                                pb[(1, j)],
                                k2[HEAD_DIM:, ts_],
                                qt[HEAD_DIM:, e, js],
                            )
                        pts = {}
                        for h in range(2):
                            for j in range(JPB):
                                pt = ptpool.tile([P, FREE], F16, tag="pt")
                                if _use_dve_exp(t, h, j):
                                    nc.vector.tensor_scalar(
                                        pt.bitcast(I16),
                                        pb[(h, j)],
                                        EXPA,
                                        EXPB,
                                        mybir.AluOpType.mult,
                                        mybir.AluOpType.add,
                                    )
                                else:
                                    nc.scalar.activation(
                                        pt,
                                        pb[(h, j)],
                                        mybir.ActivationFunctionType.Exp,
                                        bias=0.0,
                                        scale=SCALE,
                                    )
                                expctr += 1
                                pts[(h, j)] = pt
                        pending.append((t, pts))
                        if len(pending) > 2:
                            emit_D(pending.pop(0))
                    for item in pending:
                        emit_D(item)

                    # normalize: raw spill, approx-recip of the denominator
                    # rows, cross-partition DMA, multiply into `outs`
                    for h in range(2):
                        raw = nrmpool.tile([P, BLK], F32, tag=f"raw{h}")
                        if h == 0:
                            nc.scalar.copy(raw, pd[h])
                        else:
                            nc.vector.tensor_copy(raw, pd[h])
                        lo = slice(0, 64) if h == 0 else slice(64, 128)
                        hi = slice(64, 128) if h == 0 else slice(0, 64)
                        rec = nrmpool.tile([P, BLK], F32, tag="rec")
                        rec2 = nrmpool.tile([P, BLK], F32, tag="rec2")
                        # full-128-partition op: custom-DVE ops silently
                        # no-op on base_partition=64 slices; the lo-rows
                        # result (recip of raw outputs) is unused
                        nc.vector.reciprocal_approx_fast(rec, raw)
                        nc.sync.dma_start(rec2[lo, :], rec[hi, :])
                        nc.vector.tensor_mul(outs[lo, e, bs], raw[lo, :], rec2[lo, :])
                        if DEBUG and blk == 0 and e == 0 and h == 0:
                            nc.sync.dma_start(dbg["dbg_raw"][:], raw)
                            nc.sync.dma_start(dbg["dbg_rec2"][:], rec2)

            if DEBUG:
                nc.sync.dma_start(dbg["dbg_kv"][:], kv)
                nc.sync.dma_start(
                    dbg["dbg_vp"][:], vp.rearrange("p t c -> p (t c)")
                )
                nc.sync.dma_start(
                    dbg["dbg_vp2"][:], vp2.rearrange("p t c -> p (t c)")
                )
                nc.sync.dma_start(dbg["dbg_qt"][:], qt.rearrange("p e t -> p (e t)"))
                nc.sync.dma_start(
                    dbg["dbg_outs"][:], outs.rearrange("p e t -> p (e t)")
                )

            # ---------------- stage E: out-projection tail, f16 DMA out
            for c in range(NCH):
                cs = slice(c * CHK, (c + 1) * CHK)
                for m in range(DT):
                    ms = slice(m * P, (m + 1) * P)
                    py = psA.tile([P, CHK], F32, tag="pb", name="py")
                    for ee in range(ET):
                        nc.tensor.matmul(
                            py,
                            wo_sb[:, ee, ms],
                            outs[:, ee, cs],
                            start=(ee == 0),
                            stop=(ee == ET - 1),
                        )
                    ys = yspool.tile([P, CHK], F16, tag="ys", name="ys")
                    if m % 2 == 0:
                        nc.scalar.copy(ys, py)
                    else:
                        nc.vector.tensor_copy(ys, py)
                    nc.sync.dma_start(yt[ms, cs], ys)

    nc.finalize()  # Bacc: runs wait-splitting/reg-alloc passes
    return nc


_NC_CACHE = None


def _get_nc():
    global _NC_CACHE
    if _NC_CACHE is None:
        _NC_CACHE = build_bass()
    return _NC_CACHE


def shard_inputs(query, context, Wq, Wk, Wv, Wo):
    """host-side sharding: 8 cores = batch(2) x kv-group(4)"""
    in_maps = []
    xqT = [np.ascontiguousarray(query[b].T).astype(np.float16) for b in range(B)]
    xcT = [np.ascontiguousarray(context[b].T).astype(np.float16) for b in range(B)]
    for core in range(N_CORES):
        b, g = divmod(core, GROUPS)
        wqT = np.ascontiguousarray(Wq[g * DQ : (g + 1) * DQ, :].T).astype(np.float16)
        wkvT = np.ascontiguousarray(
            np.concatenate(
                [
                    Wk[g * HEAD_DIM : (g + 1) * HEAD_DIM, :],
                    Wv[g * HEAD_DIM : (g + 1) * HEAD_DIM, :],
                ],
                axis=0,
            ).T
        ).astype(np.float16)
        woT = np.ascontiguousarray(Wo[:, g * DQ : (g + 1) * DQ].T).astype(np.float16)
        in_maps.append(
            {
                "xqT": xqT[b],
                "xcT": xcT[b],
                "wqT": wqT,
                "wkvT": wkvT,
                "woT": woT,
            }
        )
    return in_maps


def kernel(query, context, Wq, Wk, Wv, Wo, _want_profile=False):
    from concourse.bass_utils import run_bass_kernel_spmd

    nc = _get_nc()
    in_maps = shard_inputs(query, context, Wq, Wk, Wv, Wo)
    res = run_bass_kernel_spmd(
        nc, in_maps, core_ids=list(range(N_CORES)), trace=_want_profile
    )
    out = np.zeros((B, TQ, D_MODEL), dtype=np.float32)
    for core in range(N_CORES):
        b = core // GROUPS
        out[b] += res.results[core]["yT"].T.astype(np.float32)
    if _want_profile:
        return out, res
    return out


# revision 25
# speedup vs baseline: 1.5316x; 1.1744x over previous
"""Cross-attention (GQA) Trainium2 Bass kernel.

Problem: B=2, Tq=Tkv=2048, D_MODEL=1024, 16 query heads / 4 kv heads,
head_dim=64.  Sharded over 8 NeuronCores as batch(2) x kv-group(4); each
core computes 4 query heads + its single kv head and a partial output
projection (Wo row-split by head group); partials are summed on host.

Dataflow (all activations kept feature-major / "transposed" on chip):

  A: qT[e,t] = WqT.T @ xqT,  kvT = WkvT.T @ xcT      (fp16, N=1024 chunks)
     v tiles built via DMA-xbar transpose of vT (no PE/DVE involvement)
  B: ST[tk,tq] = kT.T @ qT_h ; the two heads of an e-pair issue
     back-to-back as K=64 row-group-tiled matmuls -> run concurrently
  C: P = exp(ST/8): split between ScalarE (true exp LUT) and VectorE
     (1-instruction Schraudolph: fp16 bits = int16(A*s + B), ~3% elem
     error that averages out ~1/sqrt(750) under the softmax weighting)
  D: outT'[dv|sum, tq] = [v|1].T @ P ; ones-columns give the softmax
     denominators in the spare output rows for free.  D for tile t is
     emitted 2 iterations behind B(t) so the PE never waits on exp.
  norm: reciprocal_approx_fast + cross-partition DMA + multiply
  E: yT = WoT_pair.T @ outs as a dense tail phase, f16 DMA out.
"""

import os
import sys

import numpy as np

for _p in ("/opt/trn_rl_repo",):
    if _p not in sys.path and os.path.isdir(_p):
        sys.path.insert(0, _p)

import concourse.bass as bass
import concourse.bacc as bacc
import concourse.mybir as mybir
from concourse.tile import TileContext

# ---------------------------------------------------------------- problem dims
B = 2
TQ = 2048
TKV = 2048
D_MODEL = 1024
N_HEADS = 16
N_KV_HEADS = 4
HEAD_DIM = 64
N_CORES = 8
GROUPS = N_KV_HEADS  # kv groups = 4
HEADS_PER_DEV = N_HEADS // GROUPS  # 4
DQ = HEADS_PER_DEV * HEAD_DIM  # 256
DKV = 2 * HEAD_DIM  # 128 (k rows + v rows stacked)
SCALE = 1.0 / float(np.sqrt(HEAD_DIM))

P = 128
FREE = 512  # PSUM-bank-limited matmul N (fp32 out)
CHK = 512  # projection / out-proj moving chunk (PSUM one-bank limit)
BLK = 512  # tq block processed per BCD unit (1 PSUM bank per pd tile)

F32 = mybir.dt.float32
F16 = mybir.dt.float16
I16 = mybir.dt.int16

# Schraudolph exp in fp16-bit domain: f16bits(e^(s/8)) ~ int16(EXPA*s + EXPB)
EXPA = float(os.environ.get("K_EXPA", SCALE * np.log2(np.e) * 1024.0))  # 184.664
EXPB = float(os.environ.get("K_EXPB", "15301.1"))  # 15360 - 58.9 (mean-zero)

# exp-tile engine split: DVE gets |DVE_TSET|/16 of tiles.  The index is
# (t + 8j + 4h) mod 16 so every softmax row sees exactly the same
# DVE fraction (max-error is governed by the worst row), and adjacent
# t alternate engines so ScalarE/VectorE run concurrently.
DVE_TSET = frozenset(
    int(x) for x in os.environ.get("K_DVET", "1,3,5,7,9,11,13").split(",") if x != ""
)
DEBUG = os.environ.get("K_DEBUG", "0") == "1"


def _use_dve_exp(t: int, h: int) -> bool:
    # (t + 5h) mod 16: per-row-uniform DVE fraction AND within-tile
    # engine alternation (odd shift flips parity between the two heads)
    return ((t + 5 * h) % 16) in DVE_TSET


def build_bass():
    nc = bacc.Bacc()

    xq = nc.declare_dram_parameter("xqT", [D_MODEL, TQ], F16, isOutput=False)
    xc = nc.declare_dram_parameter("xcT", [D_MODEL, TKV], F16, isOutput=False)
    wq = nc.declare_dram_parameter("wqT", [D_MODEL, DQ], F16, isOutput=False)
    wkv = nc.declare_dram_parameter("wkvT", [D_MODEL, DKV], F16, isOutput=False)
    wo = nc.declare_dram_parameter("woT", [DQ, D_MODEL], F16, isOutput=False)
    yt = nc.declare_dram_parameter("yT", [D_MODEL, TQ], F16, isOutput=True)
    dbg = {}
    if DEBUG:
        for nm, shp in [
            ("dbg_kv", [P, TKV]),
            ("dbg_vp", [P, 16 * P]),
            ("dbg_vp2", [P, 16 * P]),
            ("dbg_qt", [P, 2 * TQ]),
            ("dbg_outs", [P, 2 * TQ]),
        ]:
            dbg[nm] = nc.declare_dram_parameter(nm, shp, F16, isOutput=True)
        for nm in ("dbg_raw", "dbg_rec2"):
            dbg[nm] = nc.declare_dram_parameter(nm, [P, BLK], F32, isOutput=True)

    DT = D_MODEL // P  # 8 contraction tiles
    ET = DQ // P  # 2 q-head-pair tiles
    NTK = TKV // P  # 16 tk tiles
    NBLK = TQ // BLK  # 2 tq blocks
    JPB = BLK // FREE  # 2 free-chunks per block
    NCH = TQ // CHK  # 2 chunks of 1024 for A/E

    with TileContext(nc) as tc:
        with (
            tc.tile_pool(name="consts", bufs=1) as consts,
            tc.tile_pool(name="xch", bufs=4) as xpool,
            tc.tile_pool(name="pt", bufs=12) as ptpool,
            tc.tile_pool(name="nrm", bufs=2) as nrmpool,
            tc.tile_pool(name="ys", bufs=3) as yspool,
            tc.tile_pool(name="psA", bufs=6, space="PSUM") as psA,
            tc.tile_pool(name="psB", bufs=2, space="PSUM") as psB,
        ):
            # ---------------- persistent tiles
            wq_sb = consts.tile([P, DT, DQ], F16, tag="wq")
            nc.sync.dma_start(wq_sb, wq.rearrange("(i p) e -> p i e", p=P))
            wkv_sb = consts.tile([P, DT, DKV], F16, tag="wkv")
            nc.sync.dma_start(wkv_sb, wkv.rearrange("(i p) e -> p i e", p=P))
            wo_sb = consts.tile([P, ET, D_MODEL], F16, tag="wo")
            nc.sync.dma_start(wo_sb, wo.rearrange("(i p) m -> p i m", p=P))

            qt = consts.tile([P, ET, TQ], F16, tag="qt")  # qT: head pair/tile
            kv = consts.tile([P, TKV], F16, tag="kv")  # rows 0-63 kT, 64-127 vT
            k2 = consts.tile([P, TKV], F16, tag="k2")  # rows 64-127 = kT copy
            vp = consts.tile([P, NTK, P], F16, tag="vp")  # [v | ones]
            vp2 = consts.tile([P, NTK, P], F16, tag="vp2")  # [ones | v]
            outs = consts.tile([P, ET, TQ], F16, tag="outs")  # normalized outT

            # ones halves of the [v|1] stationaries, written once
            nc.vector.memset(vp[:, :, HEAD_DIM:], 1.0)
            nc.vector.memset(vp2[:, :, :HEAD_DIM], 1.0)

            # ---------------- stage A: kv projection (4 chunks of 512,
            # i-outer / chunk-inner so each LDWEIGHTS serves 4 matmuls)
            xc_t = []
            for c in range(NCH):
                t_ = xpool.tile([P, DT, CHK], F16, tag="xch", name=f"xc{c}")
                cs = slice(c * CHK, (c + 1) * CHK)
                nc.sync.dma_start(t_, xc.rearrange("(i p) t -> p i t", p=P)[:, :, cs])
                xc_t.append(t_)
            pkv = [
                psA.tile([P, CHK], F32, tag="pb", name=f"pkv{c}") for c in range(NCH)
            ]
            for i in range(DT):
                for c in range(NCH):
                    nc.tensor.matmul(
                        pkv[c],
                        wkv_sb[:, i, :],
                        xc_t[c][:, i, :],
                        start=(i == 0),
                        stop=(i == DT - 1),
                    )
            for c in range(NCH):
                cs = slice(c * CHK, (c + 1) * CHK)
                nc.scalar.copy(kv[:, cs], pkv[c])
            # duplicate kT rows into partitions 64..127 for row-group packing
            # (Activation HWDGE queue: keeps the Sync queue free for the
            # xq chunk DMAs that gate the first BCD unit)
            nc.scalar.dma_start(k2[HEAD_DIM : 2 * HEAD_DIM, :], kv[:HEAD_DIM, :])
            # v tiles via DMA-xbar transpose, single 3D-output call:
            # vp[p, t, d] = vT[d, t*128+p]
            nc.scalar.dma_start_transpose(
                vp[:, :, :HEAD_DIM], kv[HEAD_DIM : 2 * HEAD_DIM, :]
            )
            nc.scalar.dma_start_transpose(
                vp2[:, :, HEAD_DIM:], kv[HEAD_DIM : 2 * HEAD_DIM, :]
            )

            # ---------------- stage A: q projection (chunk-pairs, e-major,
            # i-outer / chunk-inner so each LDWEIGHTS serves 2 matmuls)
            for half in range(NCH // 2):
                xq_t = []
                for c2 in range(2):
                    c = half * 2 + c2
                    t_ = xpool.tile([P, DT, CHK], F16, tag="xch", name=f"xq{c}")
                    cs = slice(c * CHK, (c + 1) * CHK)
                    nc.sync.dma_start(
                        t_, xq.rearrange("(i p) t -> p i t", p=P)[:, :, cs]
                    )
                    xq_t.append(t_)
                for e in range(ET):
                    pq = [
                        psA.tile([P, CHK], F32, tag="pb", name=f"pq{e}{c2}")
                        for c2 in range(2)
                    ]
                    for i in range(DT):
                        for c2 in range(2):
                            nc.tensor.matmul(
                                pq[c2],
                                wq_sb[:, i, e * P : (e + 1) * P],
                                xq_t[c2][:, i, :],
                                start=(i == 0),
                                stop=(i == DT - 1),
                            )
                    for c2 in range(2):
                        cs = slice(
                            (half * 2 + c2) * CHK, (half * 2 + c2 + 1) * CHK
                        )
                        nc.vector.tensor_copy(qt[:, e, cs], pq[c2])

            # ---------------- stages B/C/D, software-pipelined per tk tile
            def emit_E_chunk(c):
                cs = slice(c * CHK, (c + 1) * CHK)
                for m in range(DT):
                    ms = slice(m * P, (m + 1) * P)
                    py = psA.tile([P, CHK], F32, tag="pb", name="py")
                    for ee in range(ET):
                        nc.tensor.matmul(
                            py,
                            wo_sb[:, ee, ms],
                            outs[:, ee, cs],
                            start=(ee == 0),
                            stop=(ee == ET - 1),
                        )
                    ys = yspool.tile([P, CHK], F16, tag="ys", name="ys")
                    if m % 2 == 0:
                        nc.scalar.copy(ys, py)
                    else:
                        nc.vector.tensor_copy(ys, py)
                    nc.sync.dma_start(yt[ms, cs], ys)

            for blk in range(NBLK):
                for e in range(ET):
                    bs = slice(blk * BLK, (blk + 1) * BLK)
                    pd = [
                        psB.tile([P, BLK], F32, tag="pd", name=f"pd{_h}")
                        for _h in range(2)
                    ]

                    def emit_D(item):
                        t, pts = item
                        for h in range(2):
                            vo = vp if h == 0 else vp2
                            nc.tensor.matmul(
                                pd[h],
                                vo[:, t, :],
                                pts[h],
                                start=(t == 0),
                                stop=(t == NTK - 1),
                                skip_group_check=True,
                            )

                    pending = []
                    for t in range(NTK):
                        ts_ = slice(t * P, (t + 1) * P)
                        pb = {}
                        for h in range(2):
                            pb[h] = psA.tile([P, BLK], F32, tag="pb", name=f"pb{h}")
                        # adjacent issue => concurrent K=64 row-group pair
                        nc.tensor.matmul(
                            pb[0], kv[:HEAD_DIM, ts_], qt[:HEAD_DIM, e, bs]
                        )
                        nc.tensor.matmul(
                            pb[1], k2[HEAD_DIM:, ts_], qt[HEAD_DIM:, e, bs]
                        )
                        pts = {}
                        for h in range(2):
                            pt = ptpool.tile([P, BLK], F16, tag="pt")
                            if _use_dve_exp(t, h):
                                nc.vector.tensor_scalar(
                                    pt.bitcast(I16),
                                    pb[h],
                                    EXPA,
                                    EXPB,
                                    mybir.AluOpType.mult,
                                    mybir.AluOpType.add,
                                )
                            else:
                                nc.scalar.activation(
                                    pt,
                                    pb[h],
                                    mybir.ActivationFunctionType.Exp,
                                    bias=0.0,
                                    scale=SCALE,
                                )
                            pts[h] = pt
                        pending.append((t, pts))
                        if len(pending) > 2:
                            emit_D(pending.pop(0))
                    for item in pending:
                        emit_D(item)

                    # normalize: raw spill, approx-recip of the denominator
                    # rows, cross-partition DMA, multiply into `outs`
                    for h in range(2):
                        raw = nrmpool.tile([P, BLK], F32, tag=f"raw{h}")
                        if h == 0:
                            nc.scalar.copy(raw, pd[h])
                        else:
                            nc.vector.tensor_copy(raw, pd[h])
                        lo = slice(0, 64) if h == 0 else slice(64, 128)
                        hi = slice(64, 128) if h == 0 else slice(0, 64)
                        rec = nrmpool.tile([P, BLK], F32, tag="rec")
                        rec2 = nrmpool.tile([P, BLK], F32, tag="rec2")
                        # full-128-partition op: custom-DVE ops silently
                        # no-op on base_partition=64 slices; the lo-rows
                        # result (recip of raw outputs) is unused
                        nc.vector.reciprocal_approx_fast(rec, raw)
                        nc.sync.dma_start(rec2[lo, :], rec[hi, :])
                        nc.vector.tensor_mul(outs[lo, e, bs], raw[lo, :], rec2[lo, :])
                        if DEBUG and blk == 0 and e == 0 and h == 0:
                            nc.sync.dma_start(dbg["dbg_raw"][:], raw)
                            nc.sync.dma_start(dbg["dbg_rec2"][:], rec2)
                # blk's outs complete after e==1 normalize: interleave its
                # out-projection chunk into the next unit's BCD stream
                emit_E_chunk(blk)

            if DEBUG:
                nc.sync.dma_start(dbg["dbg_kv"][:], kv)
                nc.sync.dma_start(
                    dbg["dbg_vp"][:], vp.rearrange("p t c -> p (t c)")
                )
                nc.sync.dma_start(
                    dbg["dbg_vp2"][:], vp2.rearrange("p t c -> p (t c)")
                )
                nc.sync.dma_start(dbg["dbg_qt"][:], qt.rearrange("p e t -> p (e t)"))
                nc.sync.dma_start(
                    dbg["dbg_outs"][:], outs.rearrange("p e t -> p (e t)")
                )



    nc.finalize()  # Bacc: runs wait-splitting/reg-alloc passes
    return nc


_NC_CACHE = None


def _get_nc():
    global _NC_CACHE
    if _NC_CACHE is None:
        _NC_CACHE = build_bass()
    return _NC_CACHE


def shard_inputs(query, context, Wq, Wk, Wv, Wo):
    """host-side sharding: 8 cores = batch(2) x kv-group(4)"""
    in_maps = []
    xqT = [np.ascontiguousarray(query[b].T).astype(np.float16) for b in range(B)]
    xcT = [np.ascontiguousarray(context[b].T).astype(np.float16) for b in range(B)]
    for core in range(N_CORES):
        b, g = divmod(core, GROUPS)
        wqT = np.ascontiguousarray(Wq[g * DQ : (g + 1) * DQ, :].T).astype(np.float16)
        wkvT = np.ascontiguousarray(
            np.concatenate(
                [
                    Wk[g * HEAD_DIM : (g + 1) * HEAD_DIM, :],
                    Wv[g * HEAD_DIM : (g + 1) * HEAD_DIM, :],
                ],
                axis=0,
            ).T
        ).astype(np.float16)
        woT = np.ascontiguousarray(Wo[:, g * DQ : (g + 1) * DQ].T).astype(np.float16)
        in_maps.append(
            {
                "xqT": xqT[b],
                "xcT": xcT[b],
                "wqT": wqT,
                "wkvT": wkvT,
                "woT": woT,
            }
        )
    return in_maps


def kernel(query, context, Wq, Wk, Wv, Wo, _want_profile=False):
    from concourse.bass_utils import run_bass_kernel_spmd

    nc = _get_nc()
    in_maps = shard_inputs(query, context, Wq, Wk, Wv, Wo)
    res = run_bass_kernel_spmd(
        nc, in_maps, core_ids=list(range(N_CORES)), trace=_want_profile
    )
    out = np.zeros((B, TQ, D_MODEL), dtype=np.float32)
    for core in range(N_CORES):
        b = core // GROUPS
        out[b] += res.results[core]["yT"].T.astype(np.float32)
    if _want_profile:
        return out, res
    return out


# revision 28
# speedup vs baseline: 1.5501x; 1.0121x over previous
"""Cross-attention (GQA) Trainium2 Bass kernel.

Problem: B=2, Tq=Tkv=2048, D_MODEL=1024, 16 query heads / 4 kv heads,
head_dim=64.  Sharded over 8 NeuronCores as batch(2) x kv-group(4); each
core computes 4 query heads + its single kv head and a partial output
projection (Wo row-split by head group); partials are summed on host.

Dataflow (all activations kept feature-major / "transposed" on chip):

  A: qT[e,t] = WqT.T @ xqT,  kvT = WkvT.T @ xcT      (fp16, N=1024 chunks)
     v tiles built via DMA-xbar transpose of vT (no PE/DVE involvement)
  B: ST[tk,tq] = kT.T @ qT_h ; the two heads of an e-pair issue
     back-to-back as K=64 row-group-tiled matmuls -> run concurrently
  C: P = exp(ST/8): split between ScalarE (true exp LUT) and VectorE
     (1-instruction Schraudolph: fp16 bits = int16(A*s + B), ~3% elem
     error that averages out ~1/sqrt(750) under the softmax weighting)
  D: outT'[dv|sum, tq] = [v|1].T @ P ; ones-columns give the softmax
     denominators in the spare output rows for free.  D for tile t is
     emitted 2 iterations behind B(t) so the PE never waits on exp.
  norm: reciprocal_approx_fast + cross-partition DMA + multiply
  E: yT = WoT_pair.T @ outs as a dense tail phase, f16 DMA out.
"""

import os
import sys

import numpy as np

for _p in ("/opt/trn_rl_repo",):
    if _p not in sys.path and os.path.isdir(_p):
        sys.path.insert(0, _p)

import concourse.bass as bass
import concourse.bacc as bacc
import concourse.mybir as mybir
from concourse.tile import TileContext

# ---------------------------------------------------------------- problem dims
B = 2
TQ = 2048
TKV = 2048
D_MODEL = 1024
N_HEADS = 16
N_KV_HEADS = 4
HEAD_DIM = 64
N_CORES = 8
GROUPS = N_KV_HEADS  # kv groups = 4
HEADS_PER_DEV = N_HEADS // GROUPS  # 4
DQ = HEADS_PER_DEV * HEAD_DIM  # 256
DKV = 2 * HEAD_DIM  # 128 (k rows + v rows stacked)
SCALE = 1.0 / float(np.sqrt(HEAD_DIM))

P = 128
FREE = 512  # PSUM-bank-limited matmul N (fp32 out)
CHK = 512  # projection / out-proj moving chunk (PSUM one-bank limit)
BLK = 512  # tq block processed per BCD unit (1 PSUM bank per pd tile)

F32 = mybir.dt.float32
F16 = mybir.dt.float16
I16 = mybir.dt.int16

# Schraudolph exp in fp16-bit domain: f16bits(e^(s/8)) ~ int16(EXPA*s + EXPB)
EXPA = float(os.environ.get("K_EXPA", SCALE * np.log2(np.e) * 1024.0))  # 184.664
EXPB = float(os.environ.get("K_EXPB", "15301.1"))  # 15360 - 58.9 (mean-zero)

# exp-tile engine split: DVE gets |DVE_TSET|/16 of tiles.  The index is
# (t + 8j + 4h) mod 16 so every softmax row sees exactly the same
# DVE fraction (max-error is governed by the worst row), and adjacent
# t alternate engines so ScalarE/VectorE run concurrently.
DVE_TSET = frozenset(
    int(x) for x in os.environ.get("K_DVET", "1,3,5,7,9,11,13").split(",") if x != ""
)
DEBUG = os.environ.get("K_DEBUG", "0") == "1"


def _use_dve_exp(t: int, h: int) -> bool:
    # t-only (odd tiles): per-row-uniform DVE fraction; engines alternate
    # across consecutive t.  (This draw measured 8.4e-3 end-to-end; adding
    # an h-shift measured 12.4e-3 -- the max-error realization is
    # assignment-dependent, and t-only is the best tested.)
    return (t % 16) in DVE_TSET


def build_bass():
    nc = bacc.Bacc()

    xq = nc.declare_dram_parameter("xqT", [D_MODEL, TQ], F16, isOutput=False)
    xc = nc.declare_dram_parameter("xcT", [D_MODEL, TKV], F16, isOutput=False)
    wq = nc.declare_dram_parameter("wqT", [D_MODEL, DQ], F16, isOutput=False)
    wkv = nc.declare_dram_parameter("wkvT", [D_MODEL, DKV], F16, isOutput=False)
    wo = nc.declare_dram_parameter("woT", [DQ, D_MODEL], F16, isOutput=False)
    yt = nc.declare_dram_parameter("yT", [D_MODEL, TQ], F16, isOutput=True)
    dbg = {}
    if DEBUG:
        for nm, shp in [
            ("dbg_kv", [P, TKV]),
            ("dbg_vp", [P, 16 * P]),
            ("dbg_vp2", [P, 16 * P]),
            ("dbg_qt", [P, 2 * TQ]),
            ("dbg_outs", [P, 2 * TQ]),
        ]:
            dbg[nm] = nc.declare_dram_parameter(nm, shp, F16, isOutput=True)
        for nm in ("dbg_raw", "dbg_rec2"):
            dbg[nm] = nc.declare_dram_parameter(nm, [P, BLK], F32, isOutput=True)

    DT = D_MODEL // P  # 8 contraction tiles
    ET = DQ // P  # 2 q-head-pair tiles
    NTK = TKV // P  # 16 tk tiles
    NBLK = TQ // BLK  # 2 tq blocks
    JPB = BLK // FREE  # 2 free-chunks per block
    NCH = TQ // CHK  # 2 chunks of 1024 for A/E

    with TileContext(nc) as tc:
        with (
            tc.tile_pool(name="consts", bufs=1) as consts,
            tc.tile_pool(name="xch", bufs=4) as xpool,
            tc.tile_pool(name="pt", bufs=12) as ptpool,
            tc.tile_pool(name="nrm", bufs=2) as nrmpool,
            tc.tile_pool(name="ys", bufs=3) as yspool,
            tc.tile_pool(name="psA", bufs=6, space="PSUM") as psA,
            tc.tile_pool(name="psB", bufs=2, space="PSUM") as psB,
        ):
            # ---------------- persistent tiles (DMA order = need order:
            # wkv + xc gate the kv projection; wq + xq gate q; wo gates E)
            wkv_sb = consts.tile([P, DT, DKV], F16, tag="wkv")
            nc.sync.dma_start(wkv_sb, wkv.rearrange("(i p) e -> p i e", p=P))

            qt = consts.tile([P, ET, TQ], F16, tag="qt")  # qT: head pair/tile
            kv = consts.tile([P, TKV], F16, tag="kv")  # rows 0-63 kT, 64-127 vT
            k2 = consts.tile([P, TKV], F16, tag="k2")  # rows 64-127 = kT copy
            vp = consts.tile([P, NTK, P], F16, tag="vp")  # [v | ones]
            vp2 = consts.tile([P, NTK, P], F16, tag="vp2")  # [ones | v]
            outs = consts.tile([P, ET, TQ], F16, tag="outs")  # normalized outT

            # ones halves of the [v|1] stationaries, written once
            nc.vector.memset(vp[:, :, HEAD_DIM:], 1.0)
            nc.vector.memset(vp2[:, :, :HEAD_DIM], 1.0)

            # ---------------- stage A: kv projection, chunk-pipelined so
            # the first chunk's matmuls start as soon as its DMA lands
            xc_t = []
            for c in range(NCH):
                t_ = xpool.tile([P, DT, CHK], F16, tag="xch", name=f"xc{c}")
                cs = slice(c * CHK, (c + 1) * CHK)
                nc.sync.dma_start(t_, xc.rearrange("(i p) t -> p i t", p=P)[:, :, cs])
                xc_t.append(t_)
            wq_sb = consts.tile([P, DT, DQ], F16, tag="wq")
            nc.sync.dma_start(wq_sb, wq.rearrange("(i p) e -> p i e", p=P))
            for c in range(NCH):
                cs = slice(c * CHK, (c + 1) * CHK)
                pkv = psA.tile([P, CHK], F32, tag="pb", name=f"pkv{c}")
                for i in range(DT):
                    nc.tensor.matmul(
                        pkv,
                        wkv_sb[:, i, :],
                        xc_t[c][:, i, :],
                        start=(i == 0),
                        stop=(i == DT - 1),
                    )
                nc.scalar.copy(kv[:, cs], pkv)
                # per-chunk k-dup and v-transposes on the Activation HWDGE
                # queue (keeps Sync free for the xq DMAs gating BCD unit 0)
                nc.scalar.dma_start(
                    k2[HEAD_DIM : 2 * HEAD_DIM, cs], kv[:HEAD_DIM, cs]
                )
                tpc = NTK // NCH  # tk tiles per chunk
                nc.scalar.dma_start_transpose(
                    vp[:, c * tpc : (c + 1) * tpc, :HEAD_DIM],
                    kv[HEAD_DIM : 2 * HEAD_DIM, cs],
                )
                nc.scalar.dma_start_transpose(
                    vp2[:, c * tpc : (c + 1) * tpc, HEAD_DIM:],
                    kv[HEAD_DIM : 2 * HEAD_DIM, cs],
                )

            # ---------------- stage A: q projection emitters (chunk-pairs,
            # i-outer / chunk-inner so each LDWEIGHTS serves 2 matmuls).
            # Only chunks {0,1} are needed before BCD starts (blk = chunk);
            # the {2,3} pair is deferred into the BCD stream.
            def emit_q_pair(half):
                xq_t = []
                for c2 in range(2):
                    c = half * 2 + c2
                    t_ = xpool.tile([P, DT, CHK], F16, tag="xch", name=f"xq{c}")
                    cs = slice(c * CHK, (c + 1) * CHK)
                    nc.sync.dma_start(
                        t_, xq.rearrange("(i p) t -> p i t", p=P)[:, :, cs]
                    )
                    xq_t.append(t_)
                for e in range(ET):
                    pq = [
                        psA.tile([P, CHK], F32, tag="pb", name=f"pq{e}{c2}")
                        for c2 in range(2)
                    ]
                    for i in range(DT):
                        for c2 in range(2):
                            nc.tensor.matmul(
                                pq[c2],
                                wq_sb[:, i, e * P : (e + 1) * P],
                                xq_t[c2][:, i, :],
                                start=(i == 0),
                                stop=(i == DT - 1),
                            )
                    for c2 in range(2):
                        cs = slice(
                            (half * 2 + c2) * CHK, (half * 2 + c2 + 1) * CHK
                        )
                        nc.vector.tensor_copy(qt[:, e, cs], pq[c2])

            emit_q_pair(0)
            wo_sb = consts.tile([P, ET, D_MODEL], F16, tag="wo")
            nc.sync.dma_start(wo_sb, wo.rearrange("(i p) m -> p i m", p=P))

            # ---------------- stages B/C/D, software-pipelined per tk tile
            def emit_E_chunk(c):
                cs = slice(c * CHK, (c + 1) * CHK)
                for m in range(DT):
                    ms = slice(m * P, (m + 1) * P)
                    py = psA.tile([P, CHK], F32, tag="pb", name="py")
                    for ee in range(ET):
                        nc.tensor.matmul(
                            py,
                            wo_sb[:, ee, ms],
                            outs[:, ee, cs],
                            start=(ee == 0),
                            stop=(ee == ET - 1),
                        )
                    ys = yspool.tile([P, CHK], F16, tag="ys", name="ys")
                    if m % 2 == 0:
                        nc.scalar.copy(ys, py)
                    else:
                        nc.vector.tensor_copy(ys, py)
                    nc.sync.dma_start(yt[ms, cs], ys)

            # deferred work (E chunks, late q-proj chunks) injected one per
            # unit at t==8, so the in-order PE queue never head-of-line
            # blocks on results that are still being normalized
            deferred = []

            def emit_q_single(c):
                t_ = xpool.tile([P, DT, CHK], F16, tag="xch", name=f"xq{c}")
                cs = slice(c * CHK, (c + 1) * CHK)
                nc.sync.dma_start(
                    t_, xq.rearrange("(i p) t -> p i t", p=P)[:, :, cs]
                )
                for e in range(ET):
                    pq = psA.tile([P, CHK], F32, tag="pb", name=f"pq{c}{e}")
                    for i in range(DT):
                        nc.tensor.matmul(
                            pq,
                            wq_sb[:, i, e * P : (e + 1) * P],
                            t_[:, i, :],
                            start=(i == 0),
                            stop=(i == DT - 1),
                        )
                    nc.vector.tensor_copy(qt[:, e, cs], pq)

            for ui in range(NBLK * ET):
                blk, e = divmod(ui, ET)
                bs = slice(blk * BLK, (blk + 1) * BLK)
                pd = [
                    psB.tile([P, BLK], F32, tag="pd", name=f"pd{_h}")
                    for _h in range(2)
                ]

                def emit_D(item):
                    t, pts = item
                    for h in range(2):
                        vo = vp if h == 0 else vp2
                        nc.tensor.matmul(
                            pd[h],
                            vo[:, t, :],
                            pts[h],
                            start=(t == 0),
                            stop=(t == NTK - 1),
                            skip_group_check=True,
                        )

                pending = []
                for t in range(NTK):
                    ts_ = slice(t * P, (t + 1) * P)
                    pb = {}
                    for h in range(2):
                        pb[h] = psA.tile([P, BLK], F32, tag="pb", name=f"pb{h}")
                    # adjacent issue => concurrent K=64 row-group pair
                    nc.tensor.matmul(
                        pb[0], kv[:HEAD_DIM, ts_], qt[:HEAD_DIM, e, bs]
                    )
                    nc.tensor.matmul(
                        pb[1], k2[HEAD_DIM:, ts_], qt[HEAD_DIM:, e, bs]
                    )
                    pts = {}
                    for h in range(2):
                        pt = ptpool.tile([P, BLK], F16, tag="pt")
                        if _use_dve_exp(t, h):
                            nc.vector.tensor_scalar(
                                pt.bitcast(I16),
                                pb[h],
                                EXPA,
                                EXPB,
                                mybir.AluOpType.mult,
                                mybir.AluOpType.add,
                            )
                        else:
                            nc.scalar.activation(
                                pt,
                                pb[h],
                                mybir.ActivationFunctionType.Exp,
                                bias=0.0,
                                scale=SCALE,
                            )
                        pts[h] = pt
                    pending.append((t, pts))
                    if t == 8 and deferred:
                        deferred.pop(0)()
                    if len(pending) > 3:
                        emit_D(pending.pop(0))
                for item in pending:
                    emit_D(item)

                # normalize: raw spill, approx-recip of the denominator
                # rows, cross-partition DMA, multiply into `outs`
                for h in range(2):
                    raw = nrmpool.tile([P, BLK], F32, tag=f"raw{h}")
                    if h == 0:
                        nc.scalar.copy(raw, pd[h])
                    else:
                        nc.vector.tensor_copy(raw, pd[h])
                    lo = slice(0, 64) if h == 0 else slice(64, 128)
                    hi = slice(64, 128) if h == 0 else slice(0, 64)
                    rec = nrmpool.tile([P, BLK], F32, tag="rec")
                    rec2 = nrmpool.tile([P, BLK], F32, tag="rec2")
                    # full-128-partition op: custom-DVE ops silently
                    # no-op on base_partition=64 slices; the lo-rows
                    # result (recip of raw outputs) is unused
                    nc.vector.reciprocal_approx_fast(rec, raw)
                    nc.sync.dma_start(rec2[lo, :], rec[hi, :])
                    nc.vector.tensor_mul(outs[lo, e, bs], raw[lo, :], rec2[lo, :])
                    if DEBUG and blk == 0 and e == 0 and h == 0:
                        nc.sync.dma_start(dbg["dbg_raw"][:], raw)
                        nc.sync.dma_start(dbg["dbg_rec2"][:], rec2)
                # queue deferred work: q-proj chunks 2/3 once the pb ring
                # frees up, E chunk for this blk once both e's normalized
                if ui == 1:
                    deferred.append(lambda: emit_q_single(2))
                    deferred.append(lambda: emit_q_single(3))
                if e == ET - 1:
                    deferred.append(lambda b=blk: emit_E_chunk(b))
            for f in deferred:
                f()

            if DEBUG:
                nc.sync.dma_start(dbg["dbg_kv"][:], kv)
                nc.sync.dma_start(
                    dbg["dbg_vp"][:], vp.rearrange("p t c -> p (t c)")
                )
                nc.sync.dma_start(
                    dbg["dbg_vp2"][:], vp2.rearrange("p t c -> p (t c)")
                )
                nc.sync.dma_start(dbg["dbg_qt"][:], qt.rearrange("p e t -> p (e t)"))
                nc.sync.dma_start(
                    dbg["dbg_outs"][:], outs.rearrange("p e t -> p (e t)")
                )



    nc.finalize()  # Bacc: runs wait-splitting/reg-alloc passes
    return nc


_NC_CACHE = None


def _get_nc():
    global _NC_CACHE
    if _NC_CACHE is None:
        _NC_CACHE = build_bass()
    return _NC_CACHE


def shard_inputs(query, context, Wq, Wk, Wv, Wo):
    """host-side sharding: 8 cores = batch(2) x kv-group(4)"""
    in_maps = []
    xqT = [np.ascontiguousarray(query[b].T).astype(np.float16) for b in range(B)]
    xcT = [np.ascontiguousarray(context[b].T).astype(np.float16) for b in range(B)]
    for core in range(N_CORES):
        b, g = divmod(core, GROUPS)
        wqT = np.ascontiguousarray(Wq[g * DQ : (g + 1) * DQ, :].T).astype(np.float16)
        wkvT = np.ascontiguousarray(
            np.concatenate(
                [
                    Wk[g * HEAD_DIM : (g + 1) * HEAD_DIM, :],
                    Wv[g * HEAD_DIM : (g + 1) * HEAD_DIM, :],
                ],
                axis=0,
            ).T
        ).astype(np.float16)
        woT = np.ascontiguousarray(Wo[:, g * DQ : (g + 1) * DQ].T).astype(np.float16)
        in_maps.append(
            {
                "xqT": xqT[b],
                "xcT": xcT[b],
                "wqT": wqT,
                "wkvT": wkvT,
                "woT": woT,
            }
        )
    return in_maps


def kernel(query, context, Wq, Wk, Wv, Wo, _want_profile=False):
    from concourse.bass_utils import run_bass_kernel_spmd

    nc = _get_nc()
    in_maps = shard_inputs(query, context, Wq, Wk, Wv, Wo)
    res = run_bass_kernel_spmd(
        nc, in_maps, core_ids=list(range(N_CORES)), trace=_want_profile
    )
    out = np.zeros((B, TQ, D_MODEL), dtype=np.float32)
    for core in range(N_CORES):
        b = core // GROUPS
        out[b] += res.results[core]["yT"].T.astype(np.float32)
    if _want_profile:
        return out, res
    return out


# revision 30
# speedup vs baseline: 1.6086x; 1.0377x over previous
"""Cross-attention (GQA) Trainium2 Bass kernel.

Problem: B=2, Tq=Tkv=2048, D_MODEL=1024, 16 query heads / 4 kv heads,
head_dim=64.  Sharded over 8 NeuronCores as batch(2) x kv-group(4); each
core computes 4 query heads + its single kv head and a partial output
projection (Wo row-split by head group); partials are summed on host.

Dataflow (all activations kept feature-major / "transposed" on chip):

  A: qT[e,t] = WqT.T @ xqT,  kvT = WkvT.T @ xcT      (fp16, N=1024 chunks)
     v tiles built via DMA-xbar transpose of vT (no PE/DVE involvement)
  B: ST[tk,tq] = kT.T @ qT_h ; the two heads of an e-pair issue
     back-to-back as K=64 row-group-tiled matmuls -> run concurrently
  C: P = exp(ST/8): split between ScalarE (true exp LUT) and VectorE
     (1-instruction Schraudolph: fp16 bits = int16(A*s + B), ~3% elem
     error that averages out ~1/sqrt(750) under the softmax weighting)
  D: outT'[dv|sum, tq] = [v|1].T @ P ; ones-columns give the softmax
     denominators in the spare output rows for free.  D for tile t is
     emitted 2 iterations behind B(t) so the PE never waits on exp.
  norm: reciprocal_approx_fast + cross-partition DMA + multiply
  E: yT = WoT_pair.T @ outs as a dense tail phase, f16 DMA out.
"""

import os
import sys

import numpy as np

for _p in ("/opt/trn_rl_repo",):
    if _p not in sys.path and os.path.isdir(_p):
        sys.path.insert(0, _p)

import concourse.bass as bass
import concourse.bacc as bacc
import concourse.mybir as mybir
from concourse.tile import TileContext

# ---------------------------------------------------------------- problem dims
B = 2
TQ = 2048
TKV = 2048
D_MODEL = 1024
N_HEADS = 16
N_KV_HEADS = 4
HEAD_DIM = 64
N_CORES = 8
GROUPS = N_KV_HEADS  # kv groups = 4
HEADS_PER_DEV = N_HEADS // GROUPS  # 4
DQ = HEADS_PER_DEV * HEAD_DIM  # 256
DKV = 2 * HEAD_DIM  # 128 (k rows + v rows stacked)
SCALE = 1.0 / float(np.sqrt(HEAD_DIM))

P = 128
FREE = 512  # PSUM-bank-limited matmul N (fp32 out)
CHK = 512  # projection / out-proj moving chunk (PSUM one-bank limit)
BLK = 512  # tq block processed per BCD unit (1 PSUM bank per pd tile)

F32 = mybir.dt.float32
F16 = mybir.dt.float16
I16 = mybir.dt.int16

# Schraudolph exp in fp16-bit domain: f16bits(e^(s/8)) ~ int16(EXPA*s + EXPB)
EXPA = float(os.environ.get("K_EXPA", SCALE * np.log2(np.e) * 1024.0))  # 184.664
EXPB = float(os.environ.get("K_EXPB", "15301.1"))  # 15360 - 58.9 (mean-zero)

# exp-tile engine split: DVE gets |DVE_TSET|/16 of tiles.  The index is
# (t + 8j + 4h) mod 16 so every softmax row sees exactly the same
# DVE fraction (max-error is governed by the worst row), and adjacent
# t alternate engines so ScalarE/VectorE run concurrently.
DVE_TSET = frozenset(
    int(x) for x in os.environ.get("K_DVET", "1,3,5,7,9,11,13").split(",") if x != ""
)
DEBUG = os.environ.get("K_DEBUG", "0") == "1"


def _use_dve_exp(t: int, h: int) -> bool:
    # t-only (odd tiles): per-row-uniform DVE fraction; engines alternate
    # across consecutive t.  (This draw measured 8.4e-3 end-to-end; adding
    # an h-shift measured 12.4e-3 -- the max-error realization is
    # assignment-dependent, and t-only is the best tested.)
    return (t % 16) in DVE_TSET


def build_bass():
    nc = bacc.Bacc()

    xq = nc.declare_dram_parameter("xqT", [D_MODEL, TQ], F16, isOutput=False)
    xc = nc.declare_dram_parameter("xcT", [D_MODEL, TKV], F16, isOutput=False)
    wq = nc.declare_dram_parameter("wqT", [D_MODEL, DQ], F16, isOutput=False)
    wkv = nc.declare_dram_parameter("wkvT", [D_MODEL, DKV], F16, isOutput=False)
    wo = nc.declare_dram_parameter("woT", [DQ, D_MODEL], F16, isOutput=False)
    yt = nc.declare_dram_parameter("yT", [D_MODEL, TQ], F16, isOutput=True)
    dbg = {}
    if DEBUG:
        for nm, shp in [
            ("dbg_kv", [P, TKV]),
            ("dbg_vp", [P, 16 * P]),
            ("dbg_vp2", [P, 16 * P]),
            ("dbg_qt", [P, 2 * TQ]),
            ("dbg_outs", [P, 2 * TQ]),
        ]:
            dbg[nm] = nc.declare_dram_parameter(nm, shp, F16, isOutput=True)
        for nm in ("dbg_raw", "dbg_rec2"):
            dbg[nm] = nc.declare_dram_parameter(nm, [P, BLK], F32, isOutput=True)

    DT = D_MODEL // P  # 8 contraction tiles
    ET = DQ // P  # 2 q-head-pair tiles
    NTK = TKV // P  # 16 tk tiles
    NBLK = TQ // BLK  # 2 tq blocks
    JPB = BLK // FREE  # 2 free-chunks per block
    NCH = TQ // CHK  # 2 chunks of 1024 for A/E

    with TileContext(nc) as tc:
        with (
            tc.tile_pool(name="consts", bufs=1) as consts,
            tc.tile_pool(name="xch", bufs=4) as xpool,
            tc.tile_pool(name="pt", bufs=12) as ptpool,
            tc.tile_pool(name="nrm", bufs=2) as nrmpool,
            tc.tile_pool(name="ys", bufs=3) as yspool,
            tc.tile_pool(name="psA", bufs=6, space="PSUM") as psA,
            tc.tile_pool(name="psB", bufs=2, space="PSUM") as psB,
        ):
            # ---------------- persistent tiles (DMA order = need order:
            # wkv + xc gate the kv projection; wq + xq gate q; wo gates E)
            wkv_sb = consts.tile([P, DT, DKV], F16, tag="wkv")
            nc.sync.dma_start(wkv_sb, wkv.rearrange("(i p) e -> p i e", p=P))

            qt = consts.tile([P, ET, TQ], F16, tag="qt")  # qT: head pair/tile
            kv = consts.tile([P, TKV], F16, tag="kv")  # rows 0-63 kT, 64-127 vT
            k2 = consts.tile([P, TKV], F16, tag="k2")  # rows 64-127 = kT copy
            vp = consts.tile([P, NTK, P], F16, tag="vp")  # [v | ones]
            vp2 = consts.tile([P, NTK, P], F16, tag="vp2")  # [ones | v]
            outs = consts.tile([P, ET, TQ], F16, tag="outs")  # normalized outT

            # ones halves of the [v|1] stationaries, written once
            nc.vector.memset(vp[:, :, HEAD_DIM:], 1.0)
            nc.vector.memset(vp2[:, :, :HEAD_DIM], 1.0)

            # ---------------- stage A inputs.  xc arrives as 8 per-i-plane
            # DMAs with 4KB-contiguous lines (near-peak HBM bw); the first
            # two xq chunks ride the Activation HWDGE queue concurrently.
            wq_sb = consts.tile([P, DT, DQ], F16, tag="wq")
            nc.sync.dma_start(wq_sb, wq.rearrange("(i p) e -> p i e", p=P))
            xc_sb = consts.tile([P, DT, TKV], F16, tag="xc_sb")
            xc_r = xc.rearrange("(i p) t -> p i t", p=P)
            for i in range(DT):
                nc.sync.dma_start(xc_sb[:, i, :], xc_r[:, i, :])
            xq_r = xq.rearrange("(i p) t -> p i t", p=P)
            xq_t = {}
            for c in range(2):
                t_ = xpool.tile([P, DT, CHK], F16, tag="xch", name=f"xq{c}")
                cs = slice(c * CHK, (c + 1) * CHK)
                nc.scalar.dma_start(t_, xq_r[:, :, cs])
                xq_t[c] = t_

            # kv projection: i-outer / chunk-inner, starts on plane 0
            pkv = [
                psA.tile([P, CHK], F32, tag="pb", name=f"pkv{c}") for c in range(NCH)
            ]
            for i in range(DT):
                for c in range(NCH):
                    cs = slice(c * CHK, (c + 1) * CHK)
                    nc.tensor.matmul(
                        pkv[c],
                        wkv_sb[:, i, :],
                        xc_sb[:, i, cs],
                        start=(i == 0),
                        stop=(i == DT - 1),
                    )
            for c in range(NCH):
                cs = slice(c * CHK, (c + 1) * CHK)
                nc.scalar.copy(kv[:, cs], pkv[c])
                nc.scalar.dma_start(
                    k2[HEAD_DIM : 2 * HEAD_DIM, cs], kv[:HEAD_DIM, cs]
                )
                tpc = NTK // NCH  # tk tiles per chunk
                nc.scalar.dma_start_transpose(
                    vp[:, c * tpc : (c + 1) * tpc, :HEAD_DIM],
                    kv[HEAD_DIM : 2 * HEAD_DIM, cs],
                )
                nc.scalar.dma_start_transpose(
                    vp2[:, c * tpc : (c + 1) * tpc, HEAD_DIM:],
                    kv[HEAD_DIM : 2 * HEAD_DIM, cs],
                )

            # ---------------- stage A: q projection.  Chunks {0,1} gate the
            # first BCD units (blk = chunk); {2,3} are deferred into BCD.
            def emit_q_single(c, dma_engine=None):
                if c in xq_t:
                    t_ = xq_t[c]
                else:
                    t_ = xpool.tile([P, DT, CHK], F16, tag="xch", name=f"xq{c}")
                    cs = slice(c * CHK, (c + 1) * CHK)
                    (dma_engine or nc.sync).dma_start(t_, xq_r[:, :, cs])
                for e in range(ET):
                    pq = psA.tile([P, CHK], F32, tag="pb", name=f"pq{c}{e}")
                    for i in range(DT):
                        nc.tensor.matmul(
                            pq,
                            wq_sb[:, i, e * P : (e + 1) * P],
                            t_[:, i, :],
                            start=(i == 0),
                            stop=(i == DT - 1),
                        )
                    cs = slice(c * CHK, (c + 1) * CHK)
                    nc.vector.tensor_copy(qt[:, e, cs], pq)

            emit_q_single(0)
            emit_q_single(1)
            wo_sb = consts.tile([P, ET, D_MODEL], F16, tag="wo")
            nc.sync.dma_start(wo_sb, wo.rearrange("(i p) m -> p i m", p=P))

            # ---------------- stages B/C/D, software-pipelined per tk tile
            def emit_E_chunk(c):
                cs = slice(c * CHK, (c + 1) * CHK)
                for m in range(DT):
                    ms = slice(m * P, (m + 1) * P)
                    py = psA.tile([P, CHK], F32, tag="pb", name="py")
                    for ee in range(ET):
                        nc.tensor.matmul(
                            py,
                            wo_sb[:, ee, ms],
                            outs[:, ee, cs],
                            start=(ee == 0),
                            stop=(ee == ET - 1),
                        )
                    ys = yspool.tile([P, CHK], F16, tag="ys", name="ys")
                    if m % 2 == 0:
                        nc.scalar.copy(ys, py)
                    else:
                        nc.vector.tensor_copy(ys, py)
                    nc.sync.dma_start(yt[ms, cs], ys)

            # deferred work (E chunks, late q-proj chunks) injected one per
            # unit at t==8, so the in-order PE queue never head-of-line
            # blocks on results that are still being normalized
            deferred = []

            for ui in range(NBLK * ET):
                blk, e = divmod(ui, ET)
                bs = slice(blk * BLK, (blk + 1) * BLK)
                pd = [
                    psB.tile([P, BLK], F32, tag="pd", name=f"pd{_h}")
                    for _h in range(2)
                ]

                def emit_D(item):
                    t, pts = item
                    for h in range(2):
                        vo = vp if h == 0 else vp2
                        nc.tensor.matmul(
                            pd[h],
                            vo[:, t, :],
                            pts[h],
                            start=(t == 0),
                            stop=(t == NTK - 1),
                            skip_group_check=True,
                        )

                pending = []
                for t in range(NTK):
                    ts_ = slice(t * P, (t + 1) * P)
                    pb = {}
                    for h in range(2):
                        pb[h] = psA.tile([P, BLK], F32, tag="pb", name=f"pb{h}")
                    # adjacent issue => concurrent K=64 row-group pair
                    nc.tensor.matmul(
                        pb[0], kv[:HEAD_DIM, ts_], qt[:HEAD_DIM, e, bs]
                    )
                    nc.tensor.matmul(
                        pb[1], k2[HEAD_DIM:, ts_], qt[HEAD_DIM:, e, bs]
                    )
                    pts = {}
                    for h in range(2):
                        pt = ptpool.tile([P, BLK], F16, tag="pt")
                        if _use_dve_exp(t, h):
                            nc.vector.tensor_scalar(
                                pt.bitcast(I16),
                                pb[h],
                                EXPA,
                                EXPB,
                                mybir.AluOpType.mult,
                                mybir.AluOpType.add,
                            )
                        else:
                            nc.scalar.activation(
                                pt,
                                pb[h],
                                mybir.ActivationFunctionType.Exp,
                                bias=0.0,
                                scale=SCALE,
                            )
                        pts[h] = pt
                    pending.append((t, pts))
                    if t == 8 and deferred:
                        deferred.pop(0)()
                    if len(pending) > 3:
                        emit_D(pending.pop(0))
                for item in pending:
                    emit_D(item)

                # normalize: raw spill, approx-recip of the denominator
                # rows, cross-partition DMA, multiply into `outs`
                for h in range(2):
                    raw = nrmpool.tile([P, BLK], F32, tag=f"raw{h}")
                    if h == 0:
                        nc.scalar.copy(raw, pd[h])
                    else:
                        nc.vector.tensor_copy(raw, pd[h])
                    lo = slice(0, 64) if h == 0 else slice(64, 128)
                    hi = slice(64, 128) if h == 0 else slice(0, 64)
                    rec = nrmpool.tile([P, BLK], F32, tag="rec")
                    rec2 = nrmpool.tile([P, BLK], F32, tag="rec2")
                    # full-128-partition op: custom-DVE ops silently
                    # no-op on base_partition=64 slices; the lo-rows
                    # result (recip of raw outputs) is unused
                    nc.vector.reciprocal_approx_fast(rec, raw)
                    nc.sync.dma_start(rec2[lo, :], rec[hi, :])
                    nc.vector.tensor_mul(outs[lo, e, bs], raw[lo, :], rec2[lo, :])
                    if DEBUG and blk == 0 and e == 0 and h == 0:
                        nc.sync.dma_start(dbg["dbg_raw"][:], raw)
                        nc.sync.dma_start(dbg["dbg_rec2"][:], rec2)
                # queue deferred work: q-proj chunks 2/3 once the pb ring
                # frees up, E chunk for this blk once both e's normalized
                if ui == 1:
                    deferred.append(lambda: emit_q_single(2))
                    deferred.append(lambda: emit_q_single(3))
                if e == ET - 1:
                    deferred.append(lambda b=blk: emit_E_chunk(b))
            for f in deferred:
                f()

            if DEBUG:
                nc.sync.dma_start(dbg["dbg_kv"][:], kv)
                nc.sync.dma_start(
                    dbg["dbg_vp"][:], vp.rearrange("p t c -> p (t c)")
                )
                nc.sync.dma_start(
                    dbg["dbg_vp2"][:], vp2.rearrange("p t c -> p (t c)")
                )
                nc.sync.dma_start(dbg["dbg_qt"][:], qt.rearrange("p e t -> p (e t)"))
                nc.sync.dma_start(
                    dbg["dbg_outs"][:], outs.rearrange("p e t -> p (e t)")
                )



    nc.finalize()  # Bacc: runs wait-splitting/reg-alloc passes
    return nc


_NC_CACHE = None


def _get_nc():
    global _NC_CACHE
    if _NC_CACHE is None:
        _NC_CACHE = build_bass()
    return _NC_CACHE


def shard_inputs(query, context, Wq, Wk, Wv, Wo):
    """host-side sharding: 8 cores = batch(2) x kv-group(4)"""
    in_maps = []
    xqT = [np.ascontiguousarray(query[b].T).astype(np.float16) for b in range(B)]
    xcT = [np.ascontiguousarray(context[b].T).astype(np.float16) for b in range(B)]
    for core in range(N_CORES):
        b, g = divmod(core, GROUPS)
        wqT = np.ascontiguousarray(Wq[g * DQ : (g + 1) * DQ, :].T).astype(np.float16)
        wkvT = np.ascontiguousarray(
            np.concatenate(
                [
                    Wk[g * HEAD_DIM : (g + 1) * HEAD_DIM, :],
                    Wv[g * HEAD_DIM : (g + 1) * HEAD_DIM, :],
                ],
                axis=0,
            ).T
        ).astype(np.float16)
        woT = np.ascontiguousarray(Wo[:, g * DQ : (g + 1) * DQ].T).astype(np.float16)
        in_maps.append(
            {
                "xqT": xqT[b],
                "xcT": xcT[b],
                "wqT": wqT,
                "wkvT": wkvT,
                "woT": woT,
            }
        )
    return in_maps


def kernel(query, context, Wq, Wk, Wv, Wo, _want_profile=False):
    from concourse.bass_utils import run_bass_kernel_spmd

    nc = _get_nc()
    in_maps = shard_inputs(query, context, Wq, Wk, Wv, Wo)
    res = run_bass_kernel_spmd(
        nc, in_maps, core_ids=list(range(N_CORES)), trace=_want_profile
    )
    out = np.zeros((B, TQ, D_MODEL), dtype=np.float32)
    for core in range(N_CORES):
        b = core // GROUPS
        out[b] += res.results[core]["yT"].T.astype(np.float32)
    if _want_profile:
        return out, res
    return out


# revision 31
# speedup vs baseline: 1.7078x; 1.0617x over previous
"""Cross-attention (GQA) Trainium2 Bass kernel.

Problem: B=2, Tq=Tkv=2048, D_MODEL=1024, 16 query heads / 4 kv heads,
head_dim=64.  Sharded over 8 NeuronCores as batch(2) x kv-group(4); each
core computes 4 query heads + its single kv head and a partial output
projection (Wo row-split by head group); partials are summed on host.

Dataflow (all activations kept feature-major / "transposed" on chip):

  A: qT[e,t] = WqT.T @ xqT,  kvT = WkvT.T @ xcT      (fp16, N=1024 chunks)
     v tiles built via DMA-xbar transpose of vT (no PE/DVE involvement)
  B: ST[tk,tq] = kT.T @ qT_h ; the two heads of an e-pair issue
     back-to-back as K=64 row-group-tiled matmuls -> run concurrently
  C: P = exp(ST/8): split between ScalarE (true exp LUT) and VectorE
     (1-instruction Schraudolph: fp16 bits = int16(A*s + B), ~3% elem
     error that averages out ~1/sqrt(750) under the softmax weighting)
  D: outT'[dv|sum, tq] = [v|1].T @ P ; ones-columns give the softmax
     denominators in the spare output rows for free.  D for tile t is
     emitted 2 iterations behind B(t) so the PE never waits on exp.
  norm: reciprocal_approx_fast + cross-partition DMA + multiply
  E: yT = WoT_pair.T @ outs as a dense tail phase, f16 DMA out.
"""

import os
import sys

import numpy as np

for _p in ("/opt/trn_rl_repo",):
    if _p not in sys.path and os.path.isdir(_p):
        sys.path.insert(0, _p)

import concourse.bass as bass
import concourse.bacc as bacc
import concourse.mybir as mybir
from concourse.tile import TileContext

# ---------------------------------------------------------------- problem dims
B = 2
TQ = 2048
TKV = 2048
D_MODEL = 1024
N_HEADS = 16
N_KV_HEADS = 4
HEAD_DIM = 64
N_CORES = 8
GROUPS = N_KV_HEADS  # kv groups = 4
HEADS_PER_DEV = N_HEADS // GROUPS  # 4
DQ = HEADS_PER_DEV * HEAD_DIM  # 256
DKV = 2 * HEAD_DIM  # 128 (k rows + v rows stacked)
SCALE = 1.0 / float(np.sqrt(HEAD_DIM))

P = 128
FREE = 512  # PSUM-bank-limited matmul N (fp32 out)
CHK = 512  # projection / out-proj moving chunk (PSUM one-bank limit)
BLK = 512  # tq block processed per BCD unit (1 PSUM bank per pd tile)

F32 = mybir.dt.float32
F16 = mybir.dt.float16
I16 = mybir.dt.int16

# Schraudolph exp in fp16-bit domain: f16bits(e^(s/8)) ~ int16(EXPA*s + EXPB)
EXPA = float(os.environ.get("K_EXPA", SCALE * np.log2(np.e) * 1024.0))  # 184.664
EXPB = float(os.environ.get("K_EXPB", "15301.1"))  # 15360 - 58.9 (mean-zero)

# exp-tile engine split: DVE gets |DVE_TSET|/16 of tiles.  The index is
# (t + 8j + 4h) mod 16 so every softmax row sees exactly the same
# DVE fraction (max-error is governed by the worst row), and adjacent
# t alternate engines so ScalarE/VectorE run concurrently.
DVE_TSET = frozenset(
    int(x) for x in os.environ.get("K_DVET", "1,3,5,7,9,11,13").split(",") if x != ""
)
DEBUG = os.environ.get("K_DEBUG", "0") == "1"


def _use_dve_exp(t: int, h: int) -> bool:
    # t-only (odd tiles): per-row-uniform DVE fraction; engines alternate
    # across consecutive t.  (This draw measured 8.4e-3 end-to-end; adding
    # an h-shift measured 12.4e-3 -- the max-error realization is
    # assignment-dependent, and t-only is the best tested.)
    return (t % 16) in DVE_TSET


def build_bass():
    nc = bacc.Bacc()

    xq = nc.declare_dram_parameter("xqT", [D_MODEL, TQ], F16, isOutput=False)
    xc = nc.declare_dram_parameter("xcT", [D_MODEL, TKV], F16, isOutput=False)
    wq = nc.declare_dram_parameter("wqT", [D_MODEL, DQ], F16, isOutput=False)
    wkv = nc.declare_dram_parameter("wkvT", [D_MODEL, DKV], F16, isOutput=False)
    wo = nc.declare_dram_parameter("woT", [DQ, D_MODEL], F16, isOutput=False)
    yt = nc.declare_dram_parameter("yT", [D_MODEL, TQ], F16, isOutput=True)
    dbg = {}
    if DEBUG:
        for nm, shp in [
            ("dbg_kv", [P, TKV]),
            ("dbg_vp", [P, 16 * P]),
            ("dbg_vp2", [P, 16 * P]),
            ("dbg_qt", [P, 2 * TQ]),
            ("dbg_outs", [P, 2 * TQ]),
        ]:
            dbg[nm] = nc.declare_dram_parameter(nm, shp, F16, isOutput=True)
        for nm in ("dbg_raw", "dbg_rec2"):
            dbg[nm] = nc.declare_dram_parameter(nm, [P, BLK], F32, isOutput=True)

    DT = D_MODEL // P  # 8 contraction tiles
    ET = DQ // P  # 2 q-head-pair tiles
    NTK = TKV // P  # 16 tk tiles
    NBLK = TQ // BLK  # 2 tq blocks
    JPB = BLK // FREE  # 2 free-chunks per block
    NCH = TQ // CHK  # 2 chunks of 1024 for A/E

    with TileContext(nc) as tc:
        with (
            tc.tile_pool(name="consts", bufs=1) as consts,
            tc.tile_pool(name="xch", bufs=4) as xpool,
            tc.tile_pool(name="pt", bufs=12) as ptpool,
            tc.tile_pool(name="nrm", bufs=2) as nrmpool,
            tc.tile_pool(name="ys", bufs=3) as yspool,
            tc.tile_pool(name="psA", bufs=6, space="PSUM") as psA,
            tc.tile_pool(name="psB", bufs=2, space="PSUM") as psB,
        ):
            # ---------------- persistent tiles (DMA order = need order:
            # wkv + xc gate the kv projection; wq + xq gate q; wo gates E)
            wkv_sb = consts.tile([P, DT, DKV], F16, tag="wkv")
            nc.sync.dma_start(wkv_sb, wkv.rearrange("(i p) e -> p i e", p=P))

            qt = consts.tile([P, ET, TQ], F16, tag="qt")  # qT: head pair/tile
            kv = consts.tile([P, TKV], F16, tag="kv")  # rows 0-63 kT, 64-127 vT
            k2 = consts.tile([P, TKV], F16, tag="k2")  # rows 64-127 = kT copy
            vp = consts.tile([P, NTK, P], F16, tag="vp")  # [v | ones]
            vp2 = consts.tile([P, NTK, P], F16, tag="vp2")  # [ones | v]
            outs = consts.tile([P, ET, TQ], F16, tag="outs")  # normalized outT

            # ones halves of the [v|1] stationaries, written once
            nc.vector.memset(vp[:, :, HEAD_DIM:], 1.0)
            nc.vector.memset(vp2[:, :, :HEAD_DIM], 1.0)

            # ---------------- stage A inputs.  xc arrives as 8 per-i-plane
            # DMAs with 4KB-contiguous lines (near-peak HBM bw); the first
            # two xq chunks ride the Activation HWDGE queue concurrently.
            wq_sb = consts.tile([P, DT, DQ], F16, tag="wq")
            nc.sync.dma_start(wq_sb, wq.rearrange("(i p) e -> p i e", p=P))
            xc_sb = consts.tile([P, DT, TKV], F16, tag="xc_sb")
            xc_r = xc.rearrange("(i p) t -> p i t", p=P)
            for i in range(DT):
                nc.sync.dma_start(xc_sb[:, i, :], xc_r[:, i, :])
            xq_r = xq.rearrange("(i p) t -> p i t", p=P)
            xq_t = {}
            # only chunk 0 gates BCD unit 0 -- chunk 1 DMA is issued after
            # the kv work so it doesn't steal early HBM bandwidth
            xq_t[0] = xpool.tile([P, DT, CHK], F16, tag="xch", name="xq0")
            nc.scalar.dma_start(xq_t[0], xq_r[:, :, 0:CHK])

            # kv projection: chunk-outer so chunk c's copy + k-dup +
            # v-transpose start as soon as its 8 matmuls retire
            tpc = NTK // NCH  # tk tiles per chunk
            for c in range(NCH):
                cs = slice(c * CHK, (c + 1) * CHK)
                pkv = psA.tile([P, CHK], F32, tag="pb", name=f"pkv{c}")
                for i in range(DT):
                    nc.tensor.matmul(
                        pkv,
                        wkv_sb[:, i, :],
                        xc_sb[:, i, cs],
                        start=(i == 0),
                        stop=(i == DT - 1),
                    )
                nc.scalar.copy(kv[:, cs], pkv)
                nc.scalar.dma_start(
                    k2[HEAD_DIM : 2 * HEAD_DIM, cs], kv[:HEAD_DIM, cs]
                )
                # v-transpose on the (idle) Sync HWDGE queue
                nc.sync.dma_start_transpose(
                    vp[:, c * tpc : (c + 1) * tpc, :HEAD_DIM],
                    kv[HEAD_DIM : 2 * HEAD_DIM, cs],
                )
            xq_t[1] = xpool.tile([P, DT, CHK], F16, tag="xch", name="xq1")
            nc.scalar.dma_start(xq_t[1], xq_r[:, :, CHK : 2 * CHK])
            # vp2's v-half is a cheap same-partition DVE copy of vp's
            nc.vector.tensor_copy(vp2[:, :, HEAD_DIM:], vp[:, :, :HEAD_DIM])

            # ---------------- stage A: q projection.  Chunks {0,1} gate the
            # first BCD units (blk = chunk); {2,3} are deferred into BCD.
            def emit_q_single(c, dma_engine=None):
                if c in xq_t:
                    t_ = xq_t[c]
                else:
                    t_ = xpool.tile([P, DT, CHK], F16, tag="xch", name=f"xq{c}")
                    cs = slice(c * CHK, (c + 1) * CHK)
                    (dma_engine or nc.sync).dma_start(t_, xq_r[:, :, cs])
                for e in range(ET):
                    pq = psA.tile([P, CHK], F32, tag="pb", name=f"pq{c}{e}")
                    for i in range(DT):
                        nc.tensor.matmul(
                            pq,
                            wq_sb[:, i, e * P : (e + 1) * P],
                            t_[:, i, :],
                            start=(i == 0),
                            stop=(i == DT - 1),
                        )
                    cs = slice(c * CHK, (c + 1) * CHK)
                    nc.vector.tensor_copy(qt[:, e, cs], pq)

            emit_q_single(0)
            emit_q_single(1)
            wo_sb = consts.tile([P, ET, D_MODEL], F16, tag="wo")
            nc.sync.dma_start(wo_sb, wo.rearrange("(i p) m -> p i m", p=P))

            # ---------------- stages B/C/D, software-pipelined per tk tile
            def emit_E_chunk(c):
                cs = slice(c * CHK, (c + 1) * CHK)
                for m in range(DT):
                    ms = slice(m * P, (m + 1) * P)
                    py = psA.tile([P, CHK], F32, tag="pb", name="py")
                    for ee in range(ET):
                        nc.tensor.matmul(
                            py,
                            wo_sb[:, ee, ms],
                            outs[:, ee, cs],
                            start=(ee == 0),
                            stop=(ee == ET - 1),
                        )
                    ys = yspool.tile([P, CHK], F16, tag="ys", name="ys")
                    if m % 2 == 0:
                        nc.scalar.copy(ys, py)
                    else:
                        nc.vector.tensor_copy(ys, py)
                    nc.sync.dma_start(yt[ms, cs], ys)

            # deferred work (E chunks, late q-proj chunks) injected one per
            # unit at t==8, so the in-order PE queue never head-of-line
            # blocks on results that are still being normalized
            deferred = []

            for ui in range(NBLK * ET):
                blk, e = divmod(ui, ET)
                bs = slice(blk * BLK, (blk + 1) * BLK)
                pd = [
                    psB.tile([P, BLK], F32, tag="pd", name=f"pd{_h}")
                    for _h in range(2)
                ]

                def emit_D(item):
                    t, pts = item
                    for h in range(2):
                        vo = vp if h == 0 else vp2
                        nc.tensor.matmul(
                            pd[h],
                            vo[:, t, :],
                            pts[h],
                            start=(t == 0),
                            stop=(t == NTK - 1),
                            skip_group_check=True,
                        )

                pending = []
                for t in range(NTK):
                    ts_ = slice(t * P, (t + 1) * P)
                    pb = {}
                    for h in range(2):
                        pb[h] = psA.tile([P, BLK], F32, tag="pb", name=f"pb{h}")
                    # adjacent issue => concurrent K=64 row-group pair
                    nc.tensor.matmul(
                        pb[0], kv[:HEAD_DIM, ts_], qt[:HEAD_DIM, e, bs]
                    )
                    nc.tensor.matmul(
                        pb[1], k2[HEAD_DIM:, ts_], qt[HEAD_DIM:, e, bs]
                    )
                    pts = {}
                    for h in range(2):
                        pt = ptpool.tile([P, BLK], F16, tag="pt")
                        if _use_dve_exp(t, h):
                            nc.vector.tensor_scalar(
                                pt.bitcast(I16),
                                pb[h],
                                EXPA,
                                EXPB,
                                mybir.AluOpType.mult,
                                mybir.AluOpType.add,
                            )
                        else:
                            nc.scalar.activation(
                                pt,
                                pb[h],
                                mybir.ActivationFunctionType.Exp,
                                bias=0.0,
                                scale=SCALE,
                            )
                        pts[h] = pt
                    pending.append((t, pts))
                    if t == 8 and deferred:
                        deferred.pop(0)()
                    if len(pending) > 3:
                        emit_D(pending.pop(0))
                for item in pending:
                    emit_D(item)

                # normalize: raw spill, approx-recip of the denominator
                # rows, cross-partition DMA, multiply into `outs`
                for h in range(2):
                    raw = nrmpool.tile([P, BLK], F32, tag=f"raw{h}")
                    if h == 0:
                        nc.scalar.copy(raw, pd[h])
                    else:
                        nc.vector.tensor_copy(raw, pd[h])
                    lo = slice(0, 64) if h == 0 else slice(64, 128)
                    hi = slice(64, 128) if h == 0 else slice(0, 64)
                    rec = nrmpool.tile([P, BLK], F32, tag="rec")
                    rec2 = nrmpool.tile([P, BLK], F32, tag="rec2")
                    # full-128-partition op: custom-DVE ops silently
                    # no-op on base_partition=64 slices; the lo-rows
                    # result (recip of raw outputs) is unused
                    nc.vector.reciprocal_approx_fast(rec, raw)
                    nc.sync.dma_start(rec2[lo, :], rec[hi, :])
                    nc.vector.tensor_mul(outs[lo, e, bs], raw[lo, :], rec2[lo, :])
                    if DEBUG and blk == 0 and e == 0 and h == 0:
                        nc.sync.dma_start(dbg["dbg_raw"][:], raw)
                        nc.sync.dma_start(dbg["dbg_rec2"][:], rec2)
                # queue deferred work: q-proj chunks 2/3 once the pb ring
                # frees up, E chunk for this blk once both e's normalized
                if ui == 1:
                    deferred.append(lambda: emit_q_single(2))
                    deferred.append(lambda: emit_q_single(3))
                if e == ET - 1:
                    deferred.append(lambda b=blk: emit_E_chunk(b))
            for f in deferred:
                f()

            if DEBUG:
                nc.sync.dma_start(dbg["dbg_kv"][:], kv)
                nc.sync.dma_start(
                    dbg["dbg_vp"][:], vp.rearrange("p t c -> p (t c)")
                )
                nc.sync.dma_start(
                    dbg["dbg_vp2"][:], vp2.rearrange("p t c -> p (t c)")
                )
                nc.sync.dma_start(dbg["dbg_qt"][:], qt.rearrange("p e t -> p (e t)"))
                nc.sync.dma_start(
                    dbg["dbg_outs"][:], outs.rearrange("p e t -> p (e t)")
                )



    nc.finalize()  # Bacc: runs wait-splitting/reg-alloc passes
    return nc


_NC_CACHE = None


def _get_nc():
    global _NC_CACHE
    if _NC_CACHE is None:
        _NC_CACHE = build_bass()
    return _NC_CACHE


def shard_inputs(query, context, Wq, Wk, Wv, Wo):
    """host-side sharding: 8 cores = batch(2) x kv-group(4)"""
    in_maps = []
    xqT = [np.ascontiguousarray(query[b].T).astype(np.float16) for b in range(B)]
    xcT = [np.ascontiguousarray(context[b].T).astype(np.float16) for b in range(B)]
    for core in range(N_CORES):
        b, g = divmod(core, GROUPS)
        wqT = np.ascontiguousarray(Wq[g * DQ : (g + 1) * DQ, :].T).astype(np.float16)
        wkvT = np.ascontiguousarray(
            np.concatenate(
                [
                    Wk[g * HEAD_DIM : (g + 1) * HEAD_DIM, :],
                    Wv[g * HEAD_DIM : (g + 1) * HEAD_DIM, :],
                ],
                axis=0,
            ).T
        ).astype(np.float16)
        woT = np.ascontiguousarray(Wo[:, g * DQ : (g + 1) * DQ].T).astype(np.float16)
        in_maps.append(
            {
                "xqT": xqT[b],
                "xcT": xcT[b],
                "wqT": wqT,
                "wkvT": wkvT,
                "woT": woT,
            }
        )
    return in_maps


def kernel(query, context, Wq, Wk, Wv, Wo, _want_profile=False):
    from concourse.bass_utils import run_bass_kernel_spmd

    nc = _get_nc()
    in_maps = shard_inputs(query, context, Wq, Wk, Wv, Wo)
    res = run_bass_kernel_spmd(
        nc, in_maps, core_ids=list(range(N_CORES)), trace=_want_profile
    )
    out = np.zeros((B, TQ, D_MODEL), dtype=np.float32)
    for core in range(N_CORES):
        b = core // GROUPS
        out[b] += res.results[core]["yT"].T.astype(np.float32)
    if _want_profile:
        return out, res
    return out


# revision 35
# speedup vs baseline: 1.7162x; 1.0049x over previous
"""Cross-attention (GQA) Trainium2 Bass kernel.

Problem: B=2, Tq=Tkv=2048, D_MODEL=1024, 16 query heads / 4 kv heads,
head_dim=64.  Sharded over 8 NeuronCores as batch(2) x kv-group(4); each
core computes 4 query heads + its single kv head and a partial output
projection (Wo row-split by head group); partials are summed on host.

Dataflow (all activations kept feature-major / "transposed" on chip):

  A: qT[e,t] = WqT.T @ xqT,  kvT = WkvT.T @ xcT      (fp16, N=1024 chunks)
     v tiles built via DMA-xbar transpose of vT (no PE/DVE involvement)
  B: ST[tk,tq] = kT.T @ qT_h ; the two heads of an e-pair issue
     back-to-back as K=64 row-group-tiled matmuls -> run concurrently
  C: P = exp(ST/8): split between ScalarE (true exp LUT) and VectorE
     (1-instruction Schraudolph: fp16 bits = int16(A*s + B), ~3% elem
     error that averages out ~1/sqrt(750) under the softmax weighting)
  D: outT'[dv|sum, tq] = [v|1].T @ P ; ones-columns give the softmax
     denominators in the spare output rows for free.  D for tile t is
     emitted 2 iterations behind B(t) so the PE never waits on exp.
  norm: reciprocal_approx_fast + cross-partition DMA + multiply
  E: yT = WoT_pair.T @ outs as a dense tail phase, f16 DMA out.
"""

import os
import sys

import numpy as np

for _p in ("/opt/trn_rl_repo",):
    if _p not in sys.path and os.path.isdir(_p):
        sys.path.insert(0, _p)

import concourse.bass as bass
import concourse.bacc as bacc
import concourse.mybir as mybir
from concourse.tile import TileContext

# ---------------------------------------------------------------- problem dims
B = 2
TQ = 2048
TKV = 2048
D_MODEL = 1024
N_HEADS = 16
N_KV_HEADS = 4
HEAD_DIM = 64
N_CORES = 8
GROUPS = N_KV_HEADS  # kv groups = 4
HEADS_PER_DEV = N_HEADS // GROUPS  # 4
DQ = HEADS_PER_DEV * HEAD_DIM  # 256
DKV = 2 * HEAD_DIM  # 128 (k rows + v rows stacked)
SCALE = 1.0 / float(np.sqrt(HEAD_DIM))

P = 128
FREE = 512  # PSUM-bank-limited matmul N (fp32 out)
CHK = 512  # projection / out-proj moving chunk (PSUM one-bank limit)
BLK = 512  # tq block processed per BCD unit (1 PSUM bank per pd tile)

F32 = mybir.dt.float32
F16 = mybir.dt.float16
I16 = mybir.dt.int16

# Schraudolph exp in fp16-bit domain: f16bits(e^(s/8)) ~ int16(EXPA*s + EXPB)
EXPA = float(os.environ.get("K_EXPA", SCALE * np.log2(np.e) * 1024.0))  # 184.664
EXPB = float(os.environ.get("K_EXPB", "15301.1"))  # 15360 - 58.9 (mean-zero)

# exp-tile engine split: DVE gets |DVE_TSET|/16 of tiles.  The index is
# (t + 8j + 4h) mod 16 so every softmax row sees exactly the same
# DVE fraction (max-error is governed by the worst row), and adjacent
# t alternate engines so ScalarE/VectorE run concurrently.
DVE_TSET = frozenset(
    int(x) for x in os.environ.get("K_DVET", "1,3,5,7,9,11,13").split(",") if x != ""
)
DEBUG = os.environ.get("K_DEBUG", "0") == "1"


def _use_dve_exp(t: int, h: int) -> bool:
    # t-only (odd tiles): per-row-uniform DVE fraction; engines alternate
    # across consecutive t.  (This draw measured 8.4e-3 end-to-end; adding
    # an h-shift measured 12.4e-3 -- the max-error realization is
    # assignment-dependent, and t-only is the best tested.)
    return (t % 16) in DVE_TSET


def build_bass():
    nc = bacc.Bacc()

    xq = nc.declare_dram_parameter("xqT", [D_MODEL, TQ], F16, isOutput=False)
    xc = nc.declare_dram_parameter("xcT", [D_MODEL, TKV], F16, isOutput=False)
    wq = nc.declare_dram_parameter("wqT", [D_MODEL, DQ], F16, isOutput=False)
    wkv = nc.declare_dram_parameter("wkvT", [D_MODEL, DKV], F16, isOutput=False)
    wo = nc.declare_dram_parameter("woT", [DQ, D_MODEL], F16, isOutput=False)
    yt = nc.declare_dram_parameter("yT", [D_MODEL, TQ], F16, isOutput=True)
    dbg = {}
    if DEBUG:
        for nm, shp in [
            ("dbg_kv", [P, TKV]),
            ("dbg_vp", [P, 16 * P]),
            ("dbg_vp2", [P, 16 * P]),
            ("dbg_qt", [P, 2 * TQ]),
            ("dbg_outs", [P, 2 * TQ]),
        ]:
            dbg[nm] = nc.declare_dram_parameter(nm, shp, F16, isOutput=True)
        for nm in ("dbg_rec2",):
            dbg[nm] = nc.declare_dram_parameter(nm, [P, BLK], F32, isOutput=True)

    DT = D_MODEL // P  # 8 contraction tiles
    ET = DQ // P  # 2 q-head-pair tiles
    NTK = TKV // P  # 16 tk tiles
    NBLK = TQ // BLK  # 2 tq blocks
    JPB = BLK // FREE  # 2 free-chunks per block
    NCH = TQ // CHK  # 2 chunks of 1024 for A/E

    with TileContext(nc) as tc:
        with (
            tc.tile_pool(name="consts", bufs=1) as consts,
            tc.tile_pool(name="xch", bufs=4) as xpool,
            tc.tile_pool(name="pt", bufs=12) as ptpool,
            tc.tile_pool(name="nrm", bufs=2) as nrmpool,
            tc.tile_pool(name="ys", bufs=3) as yspool,
            tc.tile_pool(name="psA", bufs=6, space="PSUM") as psA,
            tc.tile_pool(name="psB", bufs=2, space="PSUM") as psB,
        ):
            # ---------------- persistent tiles (DMA order = need order:
            # wkv + xc gate the kv projection; wq + xq gate q; wo gates E)
            wkv_sb = consts.tile([P, DT, DKV], F16, tag="wkv")
            nc.sync.dma_start(wkv_sb, wkv.rearrange("(i p) e -> p i e", p=P))

            qt = consts.tile([P, ET, TQ], F16, tag="qt")  # qT: head pair/tile
            kv = consts.tile([P, TKV], F16, tag="kv")  # rows 0-63 kT, 64-127 vT
            k2 = consts.tile([P, TKV], F16, tag="k2")  # rows 64-127 = kT copy
            vp = consts.tile([P, NTK, P], F16, tag="vp")  # [v | ones]
            vp2 = consts.tile([P, NTK, P], F16, tag="vp2")  # [ones | v]
            outs = consts.tile([P, ET, TQ], F16, tag="outs")  # normalized outT

            # ones halves of the [v|1] stationaries, written once
            nc.vector.memset(vp[:, :, HEAD_DIM:], 1.0)
            nc.vector.memset(vp2[:, :, :HEAD_DIM], 1.0)

            # PE warmup: the input DMAs take ~12us during which the PE
            # would idle cold (HAM K=4/8 at 1.2GHz).  A burst of dummy
            # matmuls on the memset ones-tiles warms the clock gate to
            # 8/8 so stage A and the first BCD unit run at 2.4GHz.
            warm_ps = psB.tile([P, FREE], F32, tag="pd", name="warm")
            for _w in range(48):
                nc.tensor.matmul(
                    warm_ps[:HEAD_DIM, :],
                    vp[:, 0, HEAD_DIM:],
                    vp2[:, 0:8, :HEAD_DIM],
                    skip_group_check=True,
                )

            # ---------------- stage A inputs.  xc arrives as 8 per-i-plane
            # DMAs with 4KB-contiguous lines (near-peak HBM bw); the first
            # two xq chunks ride the Activation HWDGE queue concurrently.
            wq_sb = consts.tile([P, DT, DQ], F16, tag="wq")
            nc.sync.dma_start(wq_sb, wq.rearrange("(i p) e -> p i e", p=P))
            xc_sb = consts.tile([P, DT, TKV], F16, tag="xc_sb")
            xc_r = xc.rearrange("(i p) t -> p i t", p=P)
            for i in range(DT):
                nc.sync.dma_start(xc_sb[:, i, :], xc_r[:, i, :])
            xq_r = xq.rearrange("(i p) t -> p i t", p=P)
            xq_t = {}
            # only chunk 0 gates BCD unit 0 -- chunk 1 DMA is issued after
            # the kv work so it doesn't steal early HBM bandwidth
            xq_t[0] = xpool.tile([P, DT, CHK], F16, tag="xch", name="xq0")
            nc.scalar.dma_start(xq_t[0], xq_r[:, :, 0:CHK])

            # kv projection: chunk-outer so chunk c's copy + k-dup +
            # v-transpose start as soon as its 8 matmuls retire
            tpc = NTK // NCH  # tk tiles per chunk
            for c in range(NCH):
                cs = slice(c * CHK, (c + 1) * CHK)
                pkv = psA.tile([P, CHK], F32, tag="pb", name=f"pkv{c}")
                for i in range(DT):
                    nc.tensor.matmul(
                        pkv,
                        wkv_sb[:, i, :],
                        xc_sb[:, i, cs],
                        start=(i == 0),
                        stop=(i == DT - 1),
                    )
                nc.scalar.copy(kv[:, cs], pkv)
                nc.scalar.dma_start(
                    k2[HEAD_DIM : 2 * HEAD_DIM, cs], kv[:HEAD_DIM, cs]
                )
                # v-transpose on the (idle) Sync HWDGE queue
                nc.sync.dma_start_transpose(
                    vp[:, c * tpc : (c + 1) * tpc, :HEAD_DIM],
                    kv[HEAD_DIM : 2 * HEAD_DIM, cs],
                )
            xq_t[1] = xpool.tile([P, DT, CHK], F16, tag="xch", name="xq1")
            nc.scalar.dma_start(xq_t[1], xq_r[:, :, CHK : 2 * CHK])
            # vp2's v-half is a cheap same-partition DVE copy of vp's
            nc.vector.tensor_copy(vp2[:, :, HEAD_DIM:], vp[:, :, :HEAD_DIM])

            # ---------------- stage A: q projection.  Chunks {0,1} gate the
            # first BCD units (blk = chunk); {2,3} are deferred into BCD.
            def emit_q_single(c, dma_engine=None):
                if c in xq_t:
                    t_ = xq_t[c]
                else:
                    t_ = xpool.tile([P, DT, CHK], F16, tag="xch", name=f"xq{c}")
                    cs = slice(c * CHK, (c + 1) * CHK)
                    (dma_engine or nc.sync).dma_start(t_, xq_r[:, :, cs])
                for e in range(ET):
                    pq = psA.tile([P, CHK], F32, tag="pb", name=f"pq{c}{e}")
                    for i in range(DT):
                        nc.tensor.matmul(
                            pq,
                            wq_sb[:, i, e * P : (e + 1) * P],
                            t_[:, i, :],
                            start=(i == 0),
                            stop=(i == DT - 1),
                        )
                    cs = slice(c * CHK, (c + 1) * CHK)
                    nc.vector.tensor_copy(qt[:, e, cs], pq)

            emit_q_single(0)
            emit_q_single(1)
            wo_sb = consts.tile([P, ET, D_MODEL], F16, tag="wo")
            nc.sync.dma_start(wo_sb, wo.rearrange("(i p) m -> p i m", p=P))

            # ---------------- stages B/C/D, software-pipelined per tk tile
            def emit_E_chunk(c):
                cs = slice(c * CHK, (c + 1) * CHK)
                for m in range(DT):
                    ms = slice(m * P, (m + 1) * P)
                    py = psA.tile([P, CHK], F32, tag="pb", name="py")
                    for ee in range(ET):
                        nc.tensor.matmul(
                            py,
                            wo_sb[:, ee, ms],
                            outs[:, ee, cs],
                            start=(ee == 0),
                            stop=(ee == ET - 1),
                        )
                    ys = yspool.tile([P, CHK], F16, tag="ys", name="ys")
                    if m % 2 == 0:
                        nc.scalar.copy(ys, py)
                    else:
                        nc.vector.tensor_copy(ys, py)
                    nc.sync.dma_start(yt[ms, cs], ys)

            # deferred work (E chunks, late q-proj chunks) injected one per
            # unit at t==8, so the in-order PE queue never head-of-line
            # blocks on results that are still being normalized
            deferred = []

            for ui in range(NBLK * ET):
                blk, e = divmod(ui, ET)
                bs = slice(blk * BLK, (blk + 1) * BLK)
                pd = [
                    psB.tile([P, BLK], F32, tag="pd", name=f"pd{_h}")
                    for _h in range(2)
                ]

                def emit_D(item):
                    t, pts = item
                    for h in range(2):
                        vo = vp if h == 0 else vp2
                        nc.tensor.matmul(
                            pd[h],
                            vo[:, t, :],
                            pts[h],
                            start=(t == 0),
                            stop=(t == NTK - 1),
                            skip_group_check=True,
                        )

                pending = []
                for t in range(NTK):
                    ts_ = slice(t * P, (t + 1) * P)
                    pb = {}
                    for h in range(2):
                        pb[h] = psA.tile([P, BLK], F32, tag="pb", name=f"pb{h}")
                    # adjacent issue => concurrent K=64 row-group pair
                    nc.tensor.matmul(
                        pb[0], kv[:HEAD_DIM, ts_], qt[:HEAD_DIM, e, bs]
                    )
                    nc.tensor.matmul(
                        pb[1], k2[HEAD_DIM:, ts_], qt[HEAD_DIM:, e, bs]
                    )
                    pts = {}
                    for h in range(2):
                        pt = ptpool.tile([P, BLK], F16, tag="pt")
                        if _use_dve_exp(t, h):
                            nc.vector.tensor_scalar(
                                pt.bitcast(I16),
                                pb[h],
                                EXPA,
                                EXPB,
                                mybir.AluOpType.mult,
                                mybir.AluOpType.add,
                            )
                        else:
                            nc.scalar.activation(
                                pt,
                                pb[h],
                                mybir.ActivationFunctionType.Exp,
                                bias=0.0,
                                scale=SCALE,
                            )
                        pts[h] = pt
                    pending.append((t, pts))
                    if t == 8 and deferred:
                        deferred.pop(0)()
                    if len(pending) > 3:
                        emit_D(pending.pop(0))
                for item in pending:
                    emit_D(item)

                # normalize straight out of PSUM: approx-recip of the
                # denominator rows, cross-partition DMA, multiply into
                # `outs` (no SBUF spill of the raw accumulator needed)
                for h in range(2):
                    lo = slice(0, 64) if h == 0 else slice(64, 128)
                    hi = slice(64, 128) if h == 0 else slice(0, 64)
                    rec = nrmpool.tile([P, BLK], F32, tag="rec")
                    rec2 = nrmpool.tile([P, BLK], F32, tag="rec2")
                    # full-128-partition op: custom-DVE ops silently
                    # no-op on base_partition=64 slices; the lo-rows
                    # result (recip of raw outputs) is unused
                    nc.vector.reciprocal_approx_fast(rec, pd[h])
                    nc.sync.dma_start(rec2[lo, :], rec[hi, :])
                    nc.vector.tensor_mul(outs[lo, e, bs], pd[h][lo, :], rec2[lo, :])
                    if DEBUG and blk == 0 and e == 0 and h == 0:
                        nc.sync.dma_start(dbg["dbg_rec2"][:], rec2)
                # queue deferred work: q-proj chunks 2/3 once the pb ring
                # frees up, E chunk for this blk once both e's normalized
                if ui == 1:
                    deferred.append(lambda: emit_q_single(2))
                    deferred.append(lambda: emit_q_single(3))
                if e == ET - 1:
                    deferred.append(lambda b=blk: emit_E_chunk(b))
            for f in deferred:
                f()

            if DEBUG:
                nc.sync.dma_start(dbg["dbg_kv"][:], kv)
                nc.sync.dma_start(
                    dbg["dbg_vp"][:], vp.rearrange("p t c -> p (t c)")
                )
                nc.sync.dma_start(
                    dbg["dbg_vp2"][:], vp2.rearrange("p t c -> p (t c)")
                )
                nc.sync.dma_start(dbg["dbg_qt"][:], qt.rearrange("p e t -> p (e t)"))
                nc.sync.dma_start(
                    dbg["dbg_outs"][:], outs.rearrange("p e t -> p (e t)")
                )



    nc.finalize()  # Bacc: runs wait-splitting/reg-alloc passes
    return nc


_NC_CACHE = None


def _get_nc():
    global _NC_CACHE
    if _NC_CACHE is None:
        _NC_CACHE = build_bass()
    return _NC_CACHE


def shard_inputs(query, context, Wq, Wk, Wv, Wo):
    """host-side sharding: 8 cores = batch(2) x kv-group(4)"""
    in_maps = []
    xqT = [np.ascontiguousarray(query[b].T).astype(np.float16) for b in range(B)]
    xcT = [np.ascontiguousarray(context[b].T).astype(np.float16) for b in range(B)]
    for core in range(N_CORES):
        b, g = divmod(core, GROUPS)
        wqT = np.ascontiguousarray(Wq[g * DQ : (g + 1) * DQ, :].T).astype(np.float16)
        wkvT = np.ascontiguousarray(
            np.concatenate(
                [
                    Wk[g * HEAD_DIM : (g + 1) * HEAD_DIM, :],
                    Wv[g * HEAD_DIM : (g + 1) * HEAD_DIM, :],
                ],
                axis=0,
            ).T
        ).astype(np.float16)
        woT = np.ascontiguousarray(Wo[:, g * DQ : (g + 1) * DQ].T).astype(np.float16)
        in_maps.append(
            {
                "xqT": xqT[b],
                "xcT": xcT[b],
                "wqT": wqT,
                "wkvT": wkvT,
                "woT": woT,
            }
        )
    return in_maps


def kernel(query, context, Wq, Wk, Wv, Wo, _want_profile=False):
    from concourse.bass_utils import run_bass_kernel_spmd

    nc = _get_nc()
    in_maps = shard_inputs(query, context, Wq, Wk, Wv, Wo)
    res = run_bass_kernel_spmd(
        nc, in_maps, core_ids=list(range(N_CORES)), trace=_want_profile
    )
    out = np.zeros((B, TQ, D_MODEL), dtype=np.float32)
    for core in range(N_CORES):
        b = core // GROUPS
        out[b] += res.results[core]["yT"].T.astype(np.float32)
    if _want_profile:
        return out, res
    return out
